# revision 1
# baseline (speedup 1.0000x reference)
"""GCN diag-encoder (2-layer SpMM) on 8 Trainium2 NeuronCores.

Strategy: the sparse adjacency (640K edges over 10K nodes, ~0.64% dense) is
materialized as a dense A^T matrix on the host; each per-layer
  out[dst] = sum_e vals[e] * x[src[e]]        (segment-sum SpMM)
becomes dense TensorEngine matmuls.  Each core owns a 1250-wide dst slice of
A^T (padded to 1280, uint8-quantized per dst column) and streams A^T k-tiles
from HBM with an inline u8->f16 cast in the DMA, in variable-size k-tile
groups (small first/last groups shorten the pipeline ramp and tail).

Layer 1 runs A-stationary — matmul(out=psum[dst,feat], lhsT=AT_tile[src,dst],
rhs=x_tile[src,feat]) — so the layer-1 output is already node-major: the
eviction is a fused tanh+dequant-scale pass on the scalar engine (scale is
per dst node = per partition) straight into the AllGather bounce.  PSUM
accumulation groups are per 2KiB bank while layer 1 writes four 512B ranges
per bank, so each bank is seeded by one full-width start=True zero matmul.
Layer 2 (PE-bound) runs X-stationary — matmul(out=psum[feat,dst],
lhsT=x1_tile[src,feat], rhs=AT_tile[src,dst]); its dequant scale (per dst =
per free element) and the final transpose are applied on the host.

Src nodes use a padded rank-block ordering (rank r owns slots
r*1280..r*1280+1279) so layer 2's AllGathered activations line up with the
SAME A arrangement layer 1 uses — the first RESG k-tile groups of A stay
resident in SBUF for layer 2, and layer 2 interleaves resident/streamed
groups so PE starts on the earliest-arriving x1 chunks while the remaining
A-stream DMAs land.  W0 is folded into x on the host; W1 is skipped on
device when it is all-ones (torch init), else applied via a broadcast
multiply.
"""

import numpy as np
import ml_dtypes

N = 10000          # nodes
D = 128            # feature dim
NCORES = 8
S = 1250           # dst nodes per core
SP = 1280          # padded dst per core (10 tiles of 128)
KT = 80            # contraction k-tiles (padded src rows = 10240)
NPAD = KT * 128    # 10240
GSIZES = (4,) * 20   # k-tiles per group
RESG = 12          # leading groups kept resident in SBUF for layer 2
BF16 = ml_dtypes.bfloat16

_PROG_CACHE = {}


def _groups():
    out = []
    k0 = 0
    for sz in GSIZES:
        out.append((k0, k0 + sz))
        k0 += sz
    assert k0 == KT
    return out


def _build_program(nocc=False, skip=(), u8=True, resg=RESG, abufs=4,
                   w1_ones=True, l2order="streamfirst", gsizes=GSIZES):
    import concourse.bacc as bacc
    import concourse.mybir as mybir
    from concourse import tile

    f32 = mybir.dt.float32
    f16 = mybir.dt.float16
    adt = mybir.dt.uint8 if u8 else f16
    grps = []
    _k0 = 0
    for _sz in gsizes:
        grps.append((_k0, _k0 + _sz))
        _k0 += _sz
    assert _k0 == KT
    maxg = max(k1 - k0 for k0, k1 in grps)

    nc = bacc.Bacc(
        "TRN2",
        target_bir_lowering=False,
        debug=False,
        enable_asserts=False,
        num_devices=1 if nocc else NCORES,
    )

    a = nc.dram_tensor("a", [KT, 128, SP], adt, kind="ExternalInput").ap()
    # f16 copy of the streamed (non-resident) k-range: layer 2 refetches it
    # on the sync HWDGE ring, FIFO-ordered behind the critical x1 loads
    ks0 = grps[resg][0] if resg < len(grps) else KT
    af = nc.dram_tensor(
        "af", [max(KT - ks0, 1), 128, SP], f16, kind="ExternalInput"
    ).ap()
    x0 = nc.dram_tensor("x0", [128, NPAD], f16, kind="ExternalInput").ap()
    # per-dst-node dequant scales, [slot p, tile t] layout
    csc = nc.dram_tensor("csc", [128, 10], f32, kind="ExternalInput").ap()
    # broadcast W1 row (only read when not w1_ones)
    w1b = nc.dram_tensor("w1b", [128, 128], f16, kind="ExternalInput").ap()
    out = nc.dram_tensor("out", [128, SP], f32, kind="ExternalOutput").ap()

    with tile.TileContext(nc) as tc:
        with (
            tc.tile_pool(name="xp", bufs=1) as xp,
            tc.tile_pool(name="ab", bufs=abufs) as apool,
            tc.tile_pool(name="res", bufs=1) as rpool,
            tc.tile_pool(name="ev", bufs=1) as ev,
            tc.tile_pool(name="ps", bufs=1, space="PSUM") as ps,
            tc.tile_pool(name="dr", bufs=1, space="DRAM") as dr,
        ):
            # x0 is dead once layer 1 finishes; share one slot for both
            x0s = xp.tile([128, NPAD], f16, tag="xs")
            x1s = xp.tile([128, NPAD], f16, tag="xs")
            cscs = xp.tile([128, 10], f32, tag="cscs")
            w1s = xp.tile([128, 128], f16, tag="w1s")
            zl = xp.tile([128, 512], f16, tag="zl")
            warm = xp.tile([128, 1], f32, tag="warm")
            nc.scalar.dma_start(cscs[:], csc)
            if not w1_ones:
                nc.scalar.dma_start(w1s[:], w1b)
            nc.vector.memset(zl[:], 0.0)
            # pre-load the ACT tanh table so the layer-1 eviction doesn't
            # pay the table load on the critical path
            nc.scalar.activation(
                warm[:], zl[:, 0:1], mybir.ActivationFunctionType.Tanh
            )

            agin = dr.tile([128, SP], f16)
            agout = dr.tile([NCORES * 128, SP], f16, addr_space="Shared")

            res_tiles = {}

            def fetch_group(gi, halves=1, via_f16=False):
                """DMA group gi of A into an SBUF tile (f16, cast if u8)."""
                k0, k1 = grps[gi]
                nk = k1 - k0
                if gi < resg:
                    ab = rpool.tile([128, nk * SP], f16, tag=f"res{gi}")
                    res_tiles[gi] = ab
                else:
                    ab = apool.tile([128, maxg * SP], f16, tag="ab")
                if "adma" in skip:
                    nc.gpsimd.dma_start(ab[:, 0:8], a[k0][:, 0:8])
                    return ab
                bounds = [k0 + (nk * h) // halves for h in range(halves + 1)]
                for b0, b1 in zip(bounds, bounds[1:]):
                    if b0 == b1:
                        continue
                    dst = ab[:, (b0 - k0) * SP:(b1 - k0) * SP].rearrange(
                        "p (k j) -> p k j", k=b1 - b0
                    )
                    if via_f16:
                        src = af[b0 - ks0:b1 - ks0].rearrange("k p j -> p k j")
                        nc.sync.dma_start(dst, src)
                    elif u8:
                        src = a[b0:b1].rearrange("k p j -> p k j")
                        nc.gpsimd.dma_start(dst, src)
                    else:
                        src = a[b0:b1].rearrange("k p j -> p k j")
                        nc.sync.dma_start(dst, src)
                return ab

            # ---- layer 1 (A-stationary; psum is [dst slot, feat]) ----
            psum1 = ps.tile([128, SP], f32, tag="acc1")
            for c0, cn in ((0, 512), (512, 512), (1024, 256)):
                nc.tensor.matmul(
                    psum1[:, c0:c0 + cn], zl[:, 0:128], zl[:, 0:cn],
                    start=True, stop=False,
                )
            for gi, (k0, k1) in enumerate(grps):
                nc.scalar.dma_start(
                    x0s[:, k0 * 128:k1 * 128], x0[:, k0 * 128:k1 * 128]
                )
                ab = fetch_group(gi, halves=2 if gi == 0 else 1)
                if gi < len(grps) - 1:
                    for k in range(k0, k1):
                        kk = k - k0
                        rhs = x0s[:, k * 128:(k + 1) * 128]
                        for t in range(10):
                            nc.tensor.matmul(
                                psum1[:, t * 128:(t + 1) * 128],
                                ab[:, kk * SP + t * 128:
                                   kk * SP + (t + 1) * 128],
                                rhs,
                                start=False, stop=False,
                            )
                else:
                    # final group t-outer: each dst range finishes early so
                    # the tanh eviction overlaps the remaining matmuls
                    for t in range(10):
                        for k in range(k0, k1):
                            kk = k - k0
                            nc.tensor.matmul(
                                psum1[:, t * 128:(t + 1) * 128],
                                ab[:, kk * SP + t * 128:
                                   kk * SP + (t + 1) * 128],
                                x0s[:, k * 128:(k + 1) * 128],
                                start=False,
                                stop=(k == KT - 1 and t in (3, 7, 9)),
                            )

            # evict layer 1: x1 = tanh(cs_dst * psum1) [* W1] on ACT, chunked
            # agin DMA so the AllGather input lands as soon as possible.
            agin_sb = ev.tile([128, SP], f16, tag="agin")
            for t in range(10):
                c0, c1 = t * 128, (t + 1) * 128
                nc.scalar.activation(
                    agin_sb[:, c0:c1], psum1[:, c0:c1],
                    mybir.ActivationFunctionType.Tanh,
                    scale=cscs[:, t:t + 1],
                )
                if not w1_ones:
                    nc.vector.tensor_mul(
                        agin_sb[:, c0:c1], agin_sb[:, c0:c1], w1s[:]
                    )
                nc.scalar.dma_start(agin[:, c0:c1], agin_sb[:, c0:c1])

            residents_pre = list(range(resg))
            streams_pre = list(range(resg, len(grps)))
            if l2order == "streamfirst":
                _order_preview = streams_pre[:abufs] + residents_pre + streams_pre[abufs:]
            elif l2order == "weave":
                _order_preview = []
                for i in range(2):
                    if i < len(streams_pre):
                        _order_preview.append(streams_pre[i])
                    if i < len(residents_pre):
                        _order_preview.append(residents_pre[i])
                _order_preview += residents_pre[2:] + streams_pre[2:]
            else:
                _order_preview = None

            if nocc:
                nc.scalar.dma_start(agout[0:128, :], agin[:])
            else:
                nc.gpsimd.collective_compute(
                    "AllGather",
                    mybir.AluOpType.bypass,
                    replica_groups=[list(range(NCORES))],
                    ins=[agin.opt()],
                    outs=[agout.opt()],
                )
            # agout rank blocks laid side by side in the free dim are exactly
            # layer-2's lhsT tiles in the same padded rank-block order A uses.
            rank_order = []
            for gi in _order_preview:
                k0, k1 = grps[gi]
                for r in ((k0 * 128) // SP, ((k1 * 128) - 1) // SP):
                    if r not in rank_order:
                        rank_order.append(r)
            for r in range(NCORES):
                if r not in rank_order:
                    rank_order.append(r)
            for r in rank_order:
                nc.sync.dma_start(
                    x1s[:, r * SP:(r + 1) * SP],
                    agout[r * 128:(r + 1) * 128, :],
                )

            # ---- layer 2 (X-stationary; psum is [feat, dst]) ----
            # Interleave: residents first (rank-0 x1 chunk arrives first),
            # streamed groups spread out so their DMAs pipeline through the
            # abufs slots while PE chews residents.
            psum2 = ps.tile([128, SP], f32, tag="acc2")
            residents = residents_pre
            streams = streams_pre
            if l2order == "streamfirst":
                order = streams[:abufs] + residents + streams[abufs:]
            elif l2order == "weave":
                # s0 r0 s1 r1 r2 ... then remaining streams at the tail
                order = []
                for i in range(2):
                    if i < len(streams):
                        order.append(streams[i])
                    if i < len(residents):
                        order.append(residents[i])
                order += residents[2:] + streams[2:]
            else:
                order = []
                ri, si = 0, 0
                pattern = [0, 0, 1, 0, 1, 0, 1, 0, 1, 0, 1, 0]  # 1 = stream
                for p in pattern[:len(grps)]:
                    if p and si < len(streams):
                        order.append(streams[si]); si += 1
                    elif ri < len(residents):
                        order.append(residents[ri]); ri += 1
                order += residents[ri:] + streams[si:]
                if order[-1] in streams:
                    for i in range(len(order) - 2, -1, -1):
                        if order[i] in residents:
                            order.append(order.pop(i))
                            break

            ob = ev.tile([128, SP], f32, tag="ob")
            first = True
            for oi, gi in enumerate(order):
                k0, k1 = grps[gi]
                ab = (res_tiles[gi] if gi < resg
                      else fetch_group(gi, via_f16=True))
                last_grp = oi == len(order) - 1
                if not last_grp:
                    for k in range(k0, k1):
                        kk = k - k0
                        lhsT = x1s[:, k * 128:(k + 1) * 128]
                        for c0, cn in ((0, 512), (512, 512), (1024, 256)):
                            nc.tensor.matmul(
                                psum2[:, c0:c0 + cn],
                                lhsT,
                                ab[:, kk * SP + c0: kk * SP + c0 + cn],
                                start=first, stop=False,
                            )
                        first = False
                else:
                    # final group: bank-outer so each psum2 bank completes
                    # (stop=True) early and its eviction overlaps the rest
                    for c0, cn in ((0, 512), (512, 512), (1024, 256)):
                        for k in range(k0, k1):
                            kk = k - k0
                            nc.tensor.matmul(
                                psum2[:, c0:c0 + cn],
                                x1s[:, k * 128:(k + 1) * 128],
                                ab[:, kk * SP + c0: kk * SP + c0 + cn],
                                start=False, stop=(k == k1 - 1),
                            )
                        nc.vector.tensor_copy(
                            ob[:, c0:c0 + cn], psum2[:, c0:c0 + cn]
                        )
                        nc.sync.dma_start(
                            out[:, c0:c0 + cn], ob[:, c0:c0 + cn]
                        )

    nc.compile()
    return nc


def get_program(nocc=False, skip=(), u8=True, resg=RESG, abufs=4,
                w1_ones=True, l2order="streamfirst", gsizes=GSIZES):
    key = ("nc", nocc, tuple(skip), u8, resg, abufs, w1_ones, l2order,
           tuple(gsizes))
    if key not in _PROG_CACHE:
        _PROG_CACHE[key] = _build_program(nocc, skip, u8, resg, abufs,
                                          w1_ones, l2order, gsizes)
    return _PROG_CACHE[key]


def _node_perm():
    """Padded rank-block src ordering: slot i <-> (rank r = i//1280,
    local q = i%1280); global node r*1250+q for q<1250, else pad."""
    i2 = np.arange(NPAD)
    r2 = i2 // SP
    loc = i2 % SP
    node = r2 * S + loc
    valid = loc < S
    return np.where(valid, node, 0), valid


def build_in_maps(x, src, dst, vals, W, u8=True):
    """Host-side prep: dense A^T shard (u8 per-column quantized) + x0."""
    import scipy.sparse as sp

    x = np.asarray(x, np.float32)
    src = np.asarray(src, np.int64)
    dst = np.asarray(dst, np.int64)
    vals = np.asarray(vals, np.float32)
    W = np.asarray(W, np.float32)

    # A[dst, src] = sum of vals  ->  we build AT[src, dst]
    AT = sp.coo_matrix((vals, (src, dst)), shape=(N, N)).toarray()

    node2, valid2 = _node_perm()

    xw = x * W[0][None, :]
    x0p = np.zeros((NPAD, D), np.float32)
    x0p[valid2] = xw[node2[valid2]]
    x0h = np.ascontiguousarray(
        x0p.reshape(KT, 128, D).transpose(1, 0, 2).reshape(128, KT * D)
    ).astype(np.float16)

    w1brow = np.ascontiguousarray(
        np.tile(W[1][None, :], (128, 1))
    ).astype(np.float16)

    in_maps = []
    steps = []
    for c in range(NCORES):
        ATc = AT[:, c * S:(c + 1) * S]  # [N, S] float32
        colmax = np.maximum(ATc.max(axis=0), 1e-9)
        step = colmax / 255.0
        if u8:
            Aq = np.clip(np.rint(ATc * (1.0 / step)[None, :]), 0, 255).astype(
                np.uint8
            )
        else:
            Aq = (ATc * (1.0 / step)[None, :]).astype(np.float16)
        Ap = np.zeros((NPAD, SP), Aq.dtype)
        Ap[valid2, :S] = Aq[node2[valid2]]
        step_pad = np.zeros(SP, np.float32)
        step_pad[:S] = step
        steps.append(step_pad)
        # csc[p, t] = dequant scale of dst slot t*128+p
        csc_tile = np.ascontiguousarray(step_pad.reshape(10, 128).T).astype(
            np.float32
        )
        a3 = np.ascontiguousarray(Ap.reshape(KT, 128, SP))
        ks0 = sum(GSIZES[:RESG])
        in_maps.append(
            {
                "a": a3,
                "af": np.ascontiguousarray(a3[ks0:].astype(np.float16)),
                "x0": x0h,
                "csc": csc_tile,
                "w1b": w1brow,
            }
        )
    return in_maps, steps


def assemble_output(results, steps):
    outs = []
    for c in range(NCORES):
        ot = np.asarray(results[c]["out"], np.float32)  # [128, SP] feat-major
        ot = ot * steps[c][None, :]  # per-dst dequant (layer-2)
        outs.append(ot[:, :S].T)
    return np.ascontiguousarray(np.concatenate(outs, axis=0))


def kernel(x, src, dst, vals, W):
    from concourse import bass_utils

    w1_ones = bool(np.all(np.asarray(W)[1] == 1.0))
    nc = get_program(w1_ones=w1_ones)
    in_maps, steps = build_in_maps(x, src, dst, vals, W)
    # The axon terminal can wedge when a different program was loaded
    # earlier in its lifetime; after the crash the terminal restarts and a
    # retry succeeds.  Back off progressively to ride out the restart.
    import time as _time

    last_err = None
    for sleep_s in (10.0, 30.0, 60.0, 0.0):
        try:
            res = bass_utils.run_bass_kernel_spmd(
                nc, in_maps, core_ids=list(range(NCORES))
            )
            return assemble_output(res.results, steps)
        except Exception as e:  # noqa: BLE001
            last_err = e
            _time.sleep(sleep_s)
    raise last_err



# revision 34
# speedup vs baseline: 1.3784x; 1.3784x over previous
"""GCN diag-encoder (2-layer SpMM) on 8 Trainium2 NeuronCores.

Strategy: the sparse adjacency (640K edges over 10K nodes, ~0.64% dense) is
materialized as a dense A^T matrix on the host; each per-layer
  out[dst] = sum_e vals[e] * x[src[e]]        (segment-sum SpMM)
becomes dense TensorEngine matmuls.  Each core owns a 1250-wide dst slice of
A^T (padded to 1280, uint8-quantized per dst column).

v2: A^T is DMA'd ONCE as raw uint8 (half the DMA bytes of a u8->f16
cast-DMA, which is charged at the f16 destination size) and stays resident
in SBUF (100KB/partition).  The u8->f16 conversion runs on-chip, split
across the three otherwise-idle compute engines (DVE / Activation / GpSimd)
into a small rotating ring of f16 staging tiles that feed the PE.  Both
layers re-cast from the same resident u8 copy, so layer 2 needs no A
traffic at all.  This turns layer 1 from DMA-bound (~93us) into PE-bound
(~45us) and removes layer 2's 29us f16 re-stream.

Layer 1 runs A-stationary — matmul(out=psum[dst,feat], lhsT=AT_tile[src,dst],
rhs=x_tile[src,feat]) — so the layer-1 output is node-major: the eviction is
a fused tanh+dequant-scale pass on the scalar engine (scale is per dst node
= per partition) straight into the AllGather bounce.  PSUM accumulation
groups are per 2KiB bank while layer 1 writes four 512B ranges per bank, so
each bank is seeded by one full-width start=True zero matmul.  Layer 2 runs
X-stationary — matmul(out=psum[feat,dst], lhsT=x1_tile[src,feat],
rhs=AT_tile[src,dst]); its dequant scale (per dst = per free element) and
the final transpose are applied on the host.

Src nodes use a padded rank-block ordering (rank r owns slots
r*1280..r*1280+1279) so layer 2's AllGathered activations line up with the
SAME A arrangement layer 1 uses.  Layer-2 casts for the first few groups are
issued right after the AllGather so they complete during the collective
stall and PE can start the moment rank-0 activations land.  W0 is folded
into x on the host; W1 is skipped on device when it is all-ones (torch
init), else applied via a broadcast multiply.
"""

import numpy as np

N = 10000          # nodes
D = 128            # feature dim
NCORES = 8
S = 1250           # dst nodes per core
SP = 1280          # padded dst per core (10 tiles of 128)
KT = 80            # contraction k-tiles (padded src rows = 10240)
NPAD = KT * 128    # 10240
GSIZES = (1, 1, 2) + (4,) * 19   # k-tiles per group (sum = 80)
NCAST = 7          # f16 staging ring depth
NDUMMY = 40        # PE keep-warm matmuls bridging the AllGather valley
# psum bank chunks: layer-1 eviction + layer-2 column blocking
CHUNKS = ((0, 512), (512, 512), (1024, 256))

_PROG_CACHE = {}


def _build_program(nocc=False, gsizes=GSIZES, ncast=NCAST, w1_ones=True,
                   ndummy=NDUMMY):
    import concourse.bacc as bacc
    import concourse.mybir as mybir
    from bass_rust import InstructionNameOrderedSet as _NameSet
    from concourse import tile

    f32 = mybir.dt.float32
    f16 = mybir.dt.float16
    u8 = mybir.dt.uint8
    grps = []
    _k0 = 0
    for _sz in gsizes:
        grps.append((_k0, _k0 + _sz))
        _k0 += _sz
    assert _k0 == KT
    maxg = max(k1 - k0 for k0, k1 in grps)

    nc = bacc.Bacc(
        "TRN2",
        target_bir_lowering=False,
        debug=False,
        enable_asserts=False,
        num_devices=1 if nocc else NCORES,
    )

    a = nc.dram_tensor("a", [KT, 128, SP], u8, kind="ExternalInput").ap()
    x0 = nc.dram_tensor("x0", [128, NPAD], f16, kind="ExternalInput").ap()
    # per-(bank chunk, partition) dequant scales; the host sorts dst
    # columns by quant range so each (chunk, partition) slot's 4 columns
    # share one scale -> the tanh eviction is 3 bank-wide activations
    csc = nc.dram_tensor("csc", [128, 3], f32, kind="ExternalInput").ap()
    # broadcast W1 row tiled x4 (only read when not w1_ones)
    w1b = nc.dram_tensor("w1b", [128, 512], f16, kind="ExternalInput").ap()
    out = nc.dram_tensor("out", [128, SP], f32, kind="ExternalOutput").ap()

    with tile.TileContext(nc) as tc:
        with (
            tc.tile_pool(name="xp", bufs=1) as xp,
            tc.tile_pool(name="a8p", bufs=1) as a8p,
            tc.tile_pool(name="fc", bufs=ncast) as fcp,
            tc.tile_pool(name="ps", bufs=1, space="PSUM") as ps,
            tc.tile_pool(name="dr", bufs=1, space="DRAM") as dr,
        ):
            # x0 is dead once layer 1 finishes; share one slot for both
            x0s = xp.tile([128, NPAD], f16, tag="xs")
            x1s = xp.tile([128, NPAD], f16, tag="xs")
            cscs = xp.tile([128, 3], f32, tag="cscs")
            w1s = xp.tile([128, 512], f16, tag="w1s")
            zl = xp.tile([128, 512], f16, tag="zl")
            warm = xp.tile([128, 1], f32, tag="warm")
            nc.scalar.dma_start(cscs[:], csc)
            if not w1_ones:
                nc.scalar.dma_start(w1s[:], w1b)
            nc.vector.memset(zl[:], 0.0)
            # pre-load the ACT tanh table so the layer-1 eviction doesn't
            # pay the table load on the critical path
            nc.scalar.activation(
                warm[:], zl[:, 0:1], mybir.ActivationFunctionType.Tanh
            )

            agin = dr.tile([128, SP], f16)
            agout = dr.tile([NCORES * 128, SP], f16, addr_space="Shared")

            a8_tiles = {}

            def cast_group(gi, engines="vap", deps=None):
                """u8 -> f16 of resident group gi, split over the engines in
                `engines` (v=DVE, a=ACT in ~1us slices, p=POOL), shares
                proportional to their elementwise rates.  `deps` maps an
                engine letter to instruction names the slice must follow —
                used at the layer boundary so the scheduler cannot hoist
                casts ahead of the tanh -> AllGather chain."""
                k0, k1 = grps[gi]
                w = (k1 - k0) * SP
                a8 = a8_tiles[gi]
                fb = fcp.tile([128, maxg * SP], f16, tag="fc")
                rates = {"v": 4, "a": 5, "p": 3}
                tot = sum(rates[e] for e in engines)
                c0 = 0
                for e in engines:
                    c1 = w if e == engines[-1] else c0 + (w * rates[e]) // tot
                    insts = []
                    if e == "v":
                        insts.append(
                            nc.vector.tensor_copy(fb[:, c0:c1], a8[:, c0:c1])
                        )
                    elif e == "a":
                        # <=1.3k-elem slices so ACT never blocks the layer-1
                        # eviction chain behind a long copy
                        s0 = c0
                        while s0 < c1:
                            s1 = min(s0 + 1280, c1)
                            insts.append(
                                nc.scalar.copy(fb[:, s0:s1], a8[:, s0:s1])
                            )
                            s0 = s1
                    else:
                        insts.append(
                            nc.gpsimd.tensor_copy(fb[:, c0:c1], a8[:, c0:c1])
                        )
                    if deps and e in deps:
                        for inst in insts:
                            inst.ins.add_sync_dependencies_from(_NameSet(deps[e]))
                    c0 = c1
                return fb

            # ---- layer 1 (A-stationary; psum is [dst slot, feat]) ----
            # one psum tile per 2KiB bank: psum reads are dependency-tracked
            # whole-tile, so per-bank tiles let each bank's eviction start at
            # its own stop instead of after the layer's last matmul
            psum1 = []
            for ci, (c0, cn) in enumerate(CHUNKS):
                p1t = ps.tile([128, cn], f32, tag=f"acc1_{ci}", name=f"p1_{ci}")
                psum1.append(p1t)
            for ci, (c0, cn) in enumerate(CHUNKS):
                nc.tensor.matmul(
                    psum1[ci][:, 0:cn], zl[:, 0:128], zl[:, 0:cn],
                    start=True, stop=False,
                )
            # x0 for the first four groups rides ahead of their a8 loads so
            # the DMA queue can stay a couple of groups in front of the PE
            xlead = grps[3][1] * 128
            for gi, (k0, k1) in enumerate(grps):
                a8 = a8p.tile([128, (k1 - k0) * SP], u8, tag=f"a8_{gi}")
                a8_tiles[gi] = a8
                nc.sync.dma_start(
                    a8[:].rearrange("p (k j) -> p k j", k=k1 - k0),
                    a[k0:k1].rearrange("k p j -> p k j"),
                )
                if gi == 0:
                    nc.sync.dma_start(x0s[:, 0:xlead], x0[:, 0:xlead])
                if gi >= 4:
                    nc.sync.dma_start(
                        x0s[:, k0 * 128:k1 * 128], x0[:, k0 * 128:k1 * 128]
                    )
                fb = cast_group(gi)
                if gi < len(grps) - 1:
                    for k in range(k0, k1):
                        kk = k - k0
                        rhs = x0s[:, k * 128:(k + 1) * 128]
                        for t in range(10):
                            ci, tt = (t // 4, t % 4)
                            nc.tensor.matmul(
                                psum1[ci][:, tt * 128:(tt + 1) * 128],
                                fb[:, kk * SP + t * 128:
                                   kk * SP + (t + 1) * 128],
                                rhs,
                                start=False, stop=False,
                            )
                else:
                    # final group t-outer: each dst range finishes early so
                    # the tanh eviction overlaps the remaining matmuls
                    for t in range(10):
                        ci, tt = (t // 4, t % 4)
                        for k in range(k0, k1):
                            kk = k - k0
                            last_mm = nc.tensor.matmul(
                                psum1[ci][:, tt * 128:(tt + 1) * 128],
                                fb[:, kk * SP + t * 128:
                                   kk * SP + (t + 1) * 128],
                                x0s[:, k * 128:(k + 1) * 128],
                                start=False,
                                stop=(k == KT - 1 and t in (3, 7, 9)),
                            )

            # evict layer 1: x1 = tanh(cs_dst * psum1) [* W1] on ACT; DMA to
            # the AllGather bounce per psum bank so agin lands early.  The
            # whole tanh -> agin -> AllGather -> x1s chain is the only work
            # between the two PE-bound layers, so it runs at high priority
            # and its DMAs ride the otherwise-idle SP queue.
            agin_sb = xp.tile([128, SP], f16, tag="agin")
            # keep-warm matmuls: PE would otherwise idle across the AllGather
            # valley and restart cold (2.4x slower for the first 3us)
            psumd = ps.tile([128, 512], f32, tag="warmups")
            for _ in range(ndummy):
                dmm = nc.tensor.matmul(
                    psumd[:], zl[:, 0:128], zl[:, 0:512],
                    start=True, stop=True, skip_group_check=True,
                )
                # pin behind layer 1 so the scheduler cannot hoist the
                # warm-up matmuls to the (DMA-bound) start of the program
                dmm.ins.add_sync_dependencies_from(_NameSet([last_mm.ins.name]))
            tanh_last = None
            with tc.high_priority():
                for ci, (c0, cn) in enumerate(CHUNKS):
                    tanh_last = nc.scalar.activation(
                        agin_sb[:, c0:c0 + cn], psum1[ci][:, 0:cn],
                        mybir.ActivationFunctionType.Tanh,
                        scale=cscs[:, ci:ci + 1],
                    )
                    if not w1_ones:
                        nc.vector.tensor_mul(
                            agin_sb[:, c0:c0 + cn], agin_sb[:, c0:c0 + cn],
                            w1s[:, 0:cn]
                        )
                    nc.sync.dma_start(
                        agin[:, c0:c0 + cn], agin_sb[:, c0:c0 + cn]
                    )

                if nocc:
                    ag_inst = nc.sync.dma_start(agout[0:128, :], agin[:])
                else:
                    ag_inst = nc.gpsimd.collective_compute(
                        "AllGather",
                        mybir.AluOpType.bypass,
                        replica_groups=[list(range(NCORES))],
                        ins=[agin.opt()],
                        outs=[agout.opt()],
                    )
                # agout rank blocks laid side by side in the free dim are
                # exactly layer-2's lhsT tiles in the same padded rank-block
                # order A uses.  Rank 0's first k-tile lands via a tiny lead
                # DMA so PE restarts as early as possible.
                agdep = _NameSet([ag_inst.ins.name])
                x1dmas = [
                    nc.sync.dma_start(x1s[:, 0:512], agout[0:128, 0:512]),
                    nc.sync.dma_start(x1s[:, 512:SP], agout[0:128, 512:SP]),
                ] + [
                    nc.sync.dma_start(
                        x1s[:, r * SP:(r + 1) * SP],
                        agout[r * 128:(r + 1) * 128, :],
                    )
                    for r in range(1, NCORES)
                ]
                # DRAM->SBUF reads of the collective output are not tracked
                # as data deps in the single-core twin; pin them so the
                # scheduler cannot float them ahead of the agin writes
                for d in x1dmas:
                    d.ins.add_sync_dependencies_from(agdep)

            # ---- layer 2 (X-stationary; psum is [feat, dst]) ----
            # All of A is already resident as u8; only the casts re-run.
            # The first ring of casts has no x1 dependency, so it completes
            # during the AllGather and PE starts as soon as rank 0 lands.
            psum2 = []
            for ci, (c0, cn) in enumerate(CHUNKS):
                p2t = ps.tile([128, cn], f32, tag=f"acc2_{ci}", name=f"p2_{ci}")
                psum2.append(p2t)
            ob = xp.tile([128, SP], f32, tag="ob")
            first = True
            tdep = [tanh_last.ins.name]
            adep = [ag_inst.ins.name]
            for gi, (k0, k1) in enumerate(grps):
                # keep ACT free for the tanh chain and POOL free for the
                # AllGather issue while the boundary groups pre-cast on DVE;
                # ACT/POOL rejoin once their part of the chain retires
                if gi < 3:
                    fb = cast_group(gi, "v")
                elif gi < 5:
                    fb = cast_group(gi, "va", deps={"a": tdep})
                elif gi < 9:
                    fb = cast_group(gi, "vap", deps={"a": tdep, "p": adep})
                else:
                    fb = cast_group(gi)
                last_grp = gi == len(grps) - 1
                if not last_grp:
                    for k in range(k0, k1):
                        kk = k - k0
                        lhsT = x1s[:, k * 128:(k + 1) * 128]
                        for ci, (c0, cn) in enumerate(CHUNKS):
                            nc.tensor.matmul(
                                psum2[ci][:, 0:cn],
                                lhsT,
                                fb[:, kk * SP + c0: kk * SP + c0 + cn],
                                start=first, stop=False,
                            )
                        first = False
                else:
                    # final group: bank-outer with per-bank stops; ALL
                    # evictions are created after the matmuls (psum reads
                    # are tracked whole-tile, so an earlier-created read
                    # would falsely serialize the later banks' matmuls)
                    for ci, (c0, cn) in enumerate(CHUNKS):
                        for k in range(k0, k1):
                            kk = k - k0
                            nc.tensor.matmul(
                                psum2[ci][:, 0:cn],
                                x1s[:, k * 128:(k + 1) * 128],
                                fb[:, kk * SP + c0: kk * SP + c0 + cn],
                                start=False, stop=(k == k1 - 1),
                            )
                    # GPSIMD cannot read PSUM on HW: evict banks on DVE/ACT
                    dq_eng = (nc.sync, nc.scalar, nc.sync)
                    with tc.high_priority():
                        for ci, (c0, cn) in enumerate(CHUNKS):
                            if ci == 1:
                                nc.scalar.copy(
                                    ob[:, c0:c0 + cn], psum2[ci][:, 0:cn]
                                )
                            else:
                                nc.vector.tensor_copy(
                                    ob[:, c0:c0 + cn], psum2[ci][:, 0:cn]
                                )
                            dq_eng[ci].dma_start(
                                out[:, c0:c0 + cn], ob[:, c0:c0 + cn]
                            )

    nc.compile()
    return nc


def get_program(nocc=False, gsizes=GSIZES, ncast=NCAST, w1_ones=True,
                ndummy=NDUMMY):
    key = ("nc", nocc, tuple(gsizes), ncast, w1_ones, ndummy)
    if key not in _PROG_CACHE:
        _PROG_CACHE[key] = _build_program(nocc, gsizes, ncast, w1_ones,
                                          ndummy)
    return _PROG_CACHE[key]


def _slot_order():
    """Slot s = t*128 + p (tile t in 0..9, partition p) listed in quant-sort
    order: chunks of 4 (banks 0/1) or 2 (bank 2) consecutive sorted columns
    share one (chunk, partition) slot group, hence one dequant scale."""
    slots = np.empty(SP, np.int64)
    i = 0
    for ci, (tile0, ntile) in enumerate(((0, 4), (4, 4), (8, 2))):
        for p in range(128):
            for ti in range(ntile):
                slots[i] = (tile0 + ti) * 128 + p
                i += 1
    assert i == SP
    return slots


_SLOTS = _slot_order()


def _core_perm(colmax_ext):
    """perm[s] = original local dst column (or >=S for pad) in slot s, with
    columns sorted by quant range so slot groups share a scale."""
    order = np.argsort(-colmax_ext, kind="stable")  # [SP] sorted col ids
    perm = np.empty(SP, np.int64)
    perm[_SLOTS] = order
    return perm


def build_in_maps(x, src, dst, vals, W):
    """Host-side prep: dense A^T shard (u8 quantized, 4 sorted columns per
    scale group) + x0, both in the per-core permuted slot order."""
    import scipy.sparse as sp

    x = np.asarray(x, np.float32)
    src = np.asarray(src, np.int64)
    dst = np.asarray(dst, np.int64)
    vals = np.asarray(vals, np.float32)
    W = np.asarray(W, np.float32)

    # A[dst, src] = sum of vals  ->  we build AT[src, dst]
    AT = sp.coo_matrix((vals, (src, dst)), shape=(N, N)).toarray()

    # per-core column permutations (dst side of A, src rows of A, x rows)
    perms = []
    steps = []
    cscs = []
    for c in range(NCORES):
        ATc = AT[:, c * S:(c + 1) * S]  # [N, S] float32
        colmax_ext = np.full(SP, -1.0, np.float32)
        colmax_ext[:S] = ATc.max(axis=0)
        perm = _core_perm(colmax_ext)
        # group scale = max colmax over each slot group (same (chunk, p))
        cm_slot = np.maximum(colmax_ext[perm], 1e-9)  # [SP] by slot
        step_slot = np.empty(SP, np.float32)
        csc = np.empty((128, 3), np.float32)
        for ci, (tile0, ntile) in enumerate(((0, 4), (4, 4), (8, 2))):
            t_sl = slice(tile0 * 128, (tile0 + ntile) * 128)
            cm = cm_slot[t_sl].reshape(ntile, 128)    # [ntile, p]
            gmax = cm.max(axis=0) / 255.0             # [p]
            csc[:, ci] = gmax
            step_slot[t_sl] = np.tile(gmax[None, :], (ntile, 1)).reshape(-1)
        perms.append(perm)
        steps.append(step_slot)
        cscs.append(np.ascontiguousarray(csc))

    # global src slot -> node mapping using each rank's own permutation
    node2 = np.empty(NPAD, np.int64)
    valid2 = np.empty(NPAD, bool)
    for r in range(NCORES):
        pr = perms[r]
        valid = pr < S
        node2[r * SP:(r + 1) * SP] = np.where(valid, r * S + pr, 0)
        valid2[r * SP:(r + 1) * SP] = valid

    xw = x * W[0][None, :]
    x0p = np.zeros((NPAD, D), np.float32)
    x0p[valid2] = xw[node2[valid2]]
    x0h = np.ascontiguousarray(
        x0p.reshape(KT, 128, D).transpose(1, 0, 2).reshape(128, KT * D)
    ).astype(np.float16)

    w1brow = np.ascontiguousarray(
        np.tile(W[1][None, :], (128, 4))
    ).astype(np.float16)

    in_maps = []
    for c in range(NCORES):
        ATc = AT[:, c * S:(c + 1) * S]  # [N, S] float32
        perm = perms[c]
        valid = perm < S
        ATs = np.zeros((N, SP), np.float32)
        ATs[:, valid] = ATc[:, perm[valid]]           # columns in slot order
        Aq = np.clip(np.rint(ATs / steps[c][None, :]), 0, 255).astype(
            np.uint8
        )
        Ap = np.zeros((NPAD, SP), Aq.dtype)
        Ap[valid2] = Aq[node2[valid2]]                # rows in slot order
        a3 = np.ascontiguousarray(Ap.reshape(KT, 128, SP))
        in_maps.append(
            {
                "a": a3,
                "x0": x0h,
                "csc": cscs[c],
                "w1b": w1brow,
            }
        )
    return in_maps, (steps, perms)


def assemble_output(results, aux):
    steps, perms = aux
    outs = []
    for c in range(NCORES):
        ot = np.asarray(results[c]["out"], np.float32)  # [128, SP] feat-major
        ot = ot * steps[c][None, :]  # per-dst dequant (layer-2)
        perm = perms[c]
        valid = perm < S
        o = np.zeros((S, 128), np.float32)
        o[perm[valid]] = ot[:, valid].T             # un-permute dst slots
        outs.append(o)
    return np.ascontiguousarray(np.concatenate(outs, axis=0))


def kernel(x, src, dst, vals, W):
    from concourse import bass_utils

    w1_ones = bool(np.all(np.asarray(W)[1] == 1.0))
    nc = get_program(w1_ones=w1_ones)
    in_maps, steps = build_in_maps(x, src, dst, vals, W)
    # The axon terminal can wedge when a different program was loaded
    # earlier in its lifetime; after the crash the terminal restarts and a
    # retry succeeds.  Back off progressively to ride out the restart.
    import time as _time

    last_err = None
    for sleep_s in (10.0, 30.0, 60.0, 0.0):
        try:
            res = bass_utils.run_bass_kernel_spmd(
                nc, in_maps, core_ids=list(range(NCORES))
            )
            return assemble_output(res.results, steps)
        except Exception as e:  # noqa: BLE001
            last_err = e
            _time.sleep(sleep_s)
    raise last_err


# revision 39
# speedup vs baseline: 1.4404x; 1.0450x over previous
"""GCN diag-encoder (2-layer SpMM) on 8 Trainium2 NeuronCores.

Strategy: the sparse adjacency (640K edges over 10K nodes, ~0.64% dense) is
materialized as a dense A^T matrix on the host; each per-layer
  out[dst] = sum_e vals[e] * x[src[e]]        (segment-sum SpMM)
becomes dense TensorEngine matmuls.  Each core owns a 1250-wide dst slice of
A^T (padded to 1280, uint8-quantized per dst column).

v2: A^T is DMA'd ONCE as raw uint8 (half the DMA bytes of a u8->f16
cast-DMA, which is charged at the f16 destination size) and stays resident
in SBUF (100KB/partition).  The u8->f16 conversion runs on-chip, split
across the three otherwise-idle compute engines (DVE / Activation / GpSimd)
into a small rotating ring of f16 staging tiles that feed the PE.  Both
layers re-cast from the same resident u8 copy, so layer 2 needs no A
traffic at all.  This turns layer 1 from DMA-bound (~93us) into PE-bound
(~45us) and removes layer 2's 29us f16 re-stream.

Layer 1 runs A-stationary — matmul(out=psum[dst,feat], lhsT=AT_tile[src,dst],
rhs=x_tile[src,feat]) — so the layer-1 output is node-major: the eviction is
a fused tanh+dequant-scale pass on the scalar engine (scale is per dst node
= per partition) straight into the AllGather bounce.  PSUM accumulation
groups are per 2KiB bank while layer 1 writes four 512B ranges per bank, so
each bank is seeded by one full-width start=True zero matmul.  Layer 2 runs
X-stationary — matmul(out=psum[feat,dst], lhsT=x1_tile[src,feat],
rhs=AT_tile[src,dst]); its dequant scale (per dst = per free element) and
the final transpose are applied on the host.

Src nodes use a padded rank-block ordering (rank r owns slots
r*1280..r*1280+1279) so layer 2's AllGathered activations line up with the
SAME A arrangement layer 1 uses.  Layer-2 casts for the first few groups are
issued right after the AllGather so they complete during the collective
stall and PE can start the moment rank-0 activations land.  W0 is folded
into x on the host; W1 is skipped on device when it is all-ones (torch
init), else applied via a broadcast multiply.
"""

import numpy as np

N = 10000          # nodes
D = 128            # feature dim
NCORES = 8
S = 1250           # dst nodes per core
SP = 1280          # padded dst per core (10 tiles of 128)
KT = 80            # contraction k-tiles (padded src rows = 10240)
NPAD = KT * 128    # 10240
GSIZES = (1, 1, 2) + (4,) * 19   # k-tiles per group (sum = 80)
NCAST = 7          # f16 staging ring depth
NDUMMY = 6         # PE keep-warm matmuls bridging the AllGather valley
# psum bank chunks: layer-1 eviction + layer-2 column blocking
CHUNKS = ((0, 512), (512, 512), (1024, 256))

_PROG_CACHE = {}


def _build_program(nocc=False, gsizes=GSIZES, ncast=NCAST, w1_ones=True,
                   ndummy=NDUMMY):
    import concourse.bacc as bacc
    import concourse.mybir as mybir
    from bass_rust import InstructionNameOrderedSet as _NameSet
    from concourse import tile

    f32 = mybir.dt.float32
    f16 = mybir.dt.float16
    u8 = mybir.dt.uint8
    grps = []
    _k0 = 0
    for _sz in gsizes:
        grps.append((_k0, _k0 + _sz))
        _k0 += _sz
    assert _k0 == KT
    maxg = max(k1 - k0 for k0, k1 in grps)

    nc = bacc.Bacc(
        "TRN2",
        target_bir_lowering=False,
        debug=False,
        enable_asserts=False,
        num_devices=1 if nocc else NCORES,
    )

    a = nc.dram_tensor("a", [KT, 128, SP], u8, kind="ExternalInput").ap()
    x0 = nc.dram_tensor("x0", [128, NPAD], f16, kind="ExternalInput").ap()
    # per-(bank chunk, partition) dequant scales; the host sorts dst
    # columns by quant range so each (chunk, partition) slot's 4 columns
    # share one scale -> the tanh eviction is 3 bank-wide activations
    csc = nc.dram_tensor("csc", [128, 3], f32, kind="ExternalInput").ap()
    # broadcast W1 row tiled x4 (only read when not w1_ones)
    w1b = nc.dram_tensor("w1b", [128, 512], f16, kind="ExternalInput").ap()
    out = nc.dram_tensor("out", [128, SP], f32, kind="ExternalOutput").ap()

    with tile.TileContext(nc) as tc:
        with (
            tc.tile_pool(name="xp", bufs=1) as xp,
            tc.tile_pool(name="a8p", bufs=1) as a8p,
            tc.tile_pool(name="fc", bufs=ncast) as fcp,
            tc.tile_pool(name="ps", bufs=1, space="PSUM") as ps,
            tc.tile_pool(name="dr", bufs=1, space="DRAM") as dr,
        ):
            # x0 is dead once layer 1 finishes; share one slot for both
            x0s = xp.tile([128, NPAD], f16, tag="xs")
            x1s = xp.tile([128, NPAD], f16, tag="xs")
            cscs = xp.tile([128, 3], f32, tag="cscs")
            w1s = xp.tile([128, 512], f16, tag="w1s")
            zl = xp.tile([128, 512], f16, tag="zl")
            warm = xp.tile([128, 1], f32, tag="warm")
            nc.scalar.dma_start(cscs[:], csc)
            if not w1_ones:
                nc.scalar.dma_start(w1s[:], w1b)
            nc.vector.memset(zl[:], 0.0)
            # pre-load the ACT tanh table so the layer-1 eviction doesn't
            # pay the table load on the critical path
            nc.scalar.activation(
                warm[:], zl[:, 0:1], mybir.ActivationFunctionType.Tanh
            )

            agin = dr.tile([128, SP], f16)
            agout = dr.tile([NCORES * 128, SP], f16, addr_space="Shared")

            a8_tiles = {}

            def cast_group(gi, engines="vap", deps=None):
                """u8 -> f16 of resident group gi, split over the engines in
                `engines` (v=DVE, a=ACT in ~1us slices, p=POOL), shares
                proportional to their elementwise rates.  `deps` maps an
                engine letter to instruction names the slice must follow —
                used at the layer boundary so the scheduler cannot hoist
                casts ahead of the tanh -> AllGather chain."""
                k0, k1 = grps[gi]
                w = (k1 - k0) * SP
                a8 = a8_tiles[gi]
                fb = fcp.tile([128, maxg * SP], f16, tag="fc")
                rates = {"v": 4, "a": 5, "p": 3}
                tot = sum(rates[e] for e in engines)
                c0 = 0
                for e in engines:
                    c1 = w if e == engines[-1] else c0 + (w * rates[e]) // tot
                    insts = []
                    if e == "v":
                        insts.append(
                            nc.vector.tensor_copy(fb[:, c0:c1], a8[:, c0:c1])
                        )
                    elif e == "a":
                        # <=1.3k-elem slices so ACT never blocks the layer-1
                        # eviction chain behind a long copy
                        s0 = c0
                        while s0 < c1:
                            s1 = min(s0 + 1280, c1)
                            insts.append(
                                nc.scalar.copy(fb[:, s0:s1], a8[:, s0:s1])
                            )
                            s0 = s1
                    else:
                        insts.append(
                            nc.gpsimd.tensor_copy(fb[:, c0:c1], a8[:, c0:c1])
                        )
                    if deps and e in deps:
                        for inst in insts:
                            inst.ins.add_sync_dependencies_from(_NameSet(deps[e]))
                    c0 = c1
                return fb

            # ---- layer 1 (A-stationary; psum is [dst slot, feat]) ----
            # one psum tile per 2KiB bank: psum reads are dependency-tracked
            # whole-tile, so per-bank tiles let each bank's eviction start at
            # its own stop instead of after the layer's last matmul
            psum1 = []
            for ci, (c0, cn) in enumerate(CHUNKS):
                p1t = ps.tile([128, cn], f32, tag=f"acc1_{ci}", name=f"p1_{ci}")
                psum1.append(p1t)
            for ci, (c0, cn) in enumerate(CHUNKS):
                nc.tensor.matmul(
                    psum1[ci][:, 0:cn], zl[:, 0:128], zl[:, 0:cn],
                    start=True, stop=False,
                )
            # x0 for the first four groups rides ahead of their a8 loads so
            # the DMA queue can stay a couple of groups in front of the PE
            xlead = grps[3][1] * 128
            for gi, (k0, k1) in enumerate(grps):
                a8 = a8p.tile([128, (k1 - k0) * SP], u8, tag=f"a8_{gi}")
                a8_tiles[gi] = a8
                nc.sync.dma_start(
                    a8[:].rearrange("p (k j) -> p k j", k=k1 - k0),
                    a[k0:k1].rearrange("k p j -> p k j"),
                )
                if gi == 0:
                    nc.sync.dma_start(x0s[:, 0:xlead], x0[:, 0:xlead])
                if gi >= 4:
                    nc.sync.dma_start(
                        x0s[:, k0 * 128:k1 * 128], x0[:, k0 * 128:k1 * 128]
                    )
                fb = cast_group(gi)
                if gi < len(grps) - 1:
                    for k in range(k0, k1):
                        kk = k - k0
                        rhs = x0s[:, k * 128:(k + 1) * 128]
                        for t in range(10):
                            ci, tt = (t // 4, t % 4)
                            nc.tensor.matmul(
                                psum1[ci][:, tt * 128:(tt + 1) * 128],
                                fb[:, kk * SP + t * 128:
                                   kk * SP + (t + 1) * 128],
                                rhs,
                                start=False, stop=False,
                            )
                else:
                    # final group t-outer: each dst range finishes early so
                    # the tanh eviction overlaps the remaining matmuls
                    for t in range(10):
                        ci, tt = (t // 4, t % 4)
                        for k in range(k0, k1):
                            kk = k - k0
                            last_mm = nc.tensor.matmul(
                                psum1[ci][:, tt * 128:(tt + 1) * 128],
                                fb[:, kk * SP + t * 128:
                                   kk * SP + (t + 1) * 128],
                                x0s[:, k * 128:(k + 1) * 128],
                                start=False,
                                stop=(k == KT - 1 and t in (3, 7, 9)),
                            )

            # evict layer 1: x1 = tanh(cs_dst * psum1) [* W1] on ACT; DMA to
            # the AllGather bounce per psum bank so agin lands early.  The
            # whole tanh -> agin -> AllGather -> x1s chain is the only work
            # between the two PE-bound layers, so it runs at high priority
            # and its DMAs ride the otherwise-idle SP queue.
            agin_sb = xp.tile([128, SP], f16, tag="agin")
            # keep-warm matmuls: PE would otherwise idle across the AllGather
            # valley and restart cold (2.4x slower for the first 3us)
            psumd = ps.tile([128, 512], f32, tag="warmups")
            for _ in range(ndummy):
                dmm = nc.tensor.matmul(
                    psumd[:], zl[:, 0:128], zl[:, 0:512],
                    start=True, stop=True, skip_group_check=True,
                )
                # pin behind layer 1 so the scheduler cannot hoist the
                # warm-up matmuls to the (DMA-bound) start of the program
                dmm.ins.add_sync_dependencies_from(_NameSet([last_mm.ins.name]))
            tanh_last = None
            with tc.high_priority():
                for ci, (c0, cn) in enumerate(CHUNKS):
                    tanh_last = nc.scalar.activation(
                        agin_sb[:, c0:c0 + cn], psum1[ci][:, 0:cn],
                        mybir.ActivationFunctionType.Tanh,
                        scale=cscs[:, ci:ci + 1],
                    )
                    if not w1_ones:
                        nc.vector.tensor_mul(
                            agin_sb[:, c0:c0 + cn], agin_sb[:, c0:c0 + cn],
                            w1s[:, 0:cn]
                        )
                    nc.sync.dma_start(
                        agin[:, c0:c0 + cn], agin_sb[:, c0:c0 + cn]
                    )

                if nocc:
                    ag_inst = nc.sync.dma_start(agout[0:128, :], agin[:])
                else:
                    ag_inst = nc.gpsimd.collective_compute(
                        "AllGather",
                        mybir.AluOpType.bypass,
                        replica_groups=[list(range(NCORES))],
                        ins=[agin.opt()],
                        outs=[agout.opt()],
                    )
                # A's src row-blocks are rotated per core so block 0 is the
                # core's OWN rank: layer 2's first 10 k-tiles read agin_sb
                # directly (no AllGather round-trip), and block i (i>=1) is
                # rank (pid+i)%8, fetched from agout at a register-computed
                # offset.
                agdep = _NameSet([ag_inst.ins.name])
                pid = nc.sync.partition_id()
                x1dmas = []
                for i in range(1, NCORES):
                    boff = ((pid + i) % NCORES) * (128 * SP)
                    if i == 1:
                        # lead slice so k-tile 10 can start while the rest
                        # of the block is in flight
                        dsrc = agout[0:128, 0:384].copy()
                        dsrc.offset = boff
                        x1dmas.append(
                            nc.sync.dma_start(x1s[:, SP:SP + 384], dsrc)
                        )
                        dsrc = agout[0:128, 384:SP].copy()
                        dsrc.offset = boff + 384
                        x1dmas.append(
                            nc.sync.dma_start(
                                x1s[:, SP + 384:2 * SP], dsrc
                            )
                        )
                    else:
                        dsrc = agout[0:128, :].copy()
                        dsrc.offset = boff
                        x1dmas.append(
                            nc.sync.dma_start(
                                x1s[:, i * SP:(i + 1) * SP], dsrc
                            )
                        )
                # DRAM->SBUF reads of the collective output are not tracked
                # as data deps in the single-core twin; pin them so the
                # scheduler cannot float them ahead of the agin writes
                for d in x1dmas:
                    d.ins.add_sync_dependencies_from(agdep)

            # ---- layer 2 (X-stationary; psum is [feat, dst]) ----
            # All of A is already resident as u8; only the casts re-run.
            # The first ring of casts has no x1 dependency, so it completes
            # during the AllGather and PE starts as soon as rank 0 lands.
            psum2 = []
            for ci, (c0, cn) in enumerate(CHUNKS):
                p2t = ps.tile([128, cn], f32, tag=f"acc2_{ci}", name=f"p2_{ci}")
                psum2.append(p2t)
            ob = xp.tile([128, SP], f32, tag="ob")

            def lhsT_of(k):
                # row-block 0 is the core's own rank: its activations are
                # already on-chip in agin_sb (same [dst slot, feat] layout)
                if k < 10:
                    return agin_sb[:, k * 128:(k + 1) * 128]
                return x1s[:, k * 128:(k + 1) * 128]

            first = True
            tdep = [tanh_last.ins.name]
            adep = [ag_inst.ins.name]
            for gi, (k0, k1) in enumerate(grps):
                # keep ACT free for the tanh chain and POOL free for the
                # AllGather issue while the boundary groups pre-cast on DVE;
                # ACT/POOL rejoin once their part of the chain retires
                if gi < 3:
                    fb = cast_group(gi, "v")
                elif gi < 5:
                    fb = cast_group(gi, "va", deps={"a": tdep})
                elif gi < 9:
                    fb = cast_group(gi, "vap", deps={"a": tdep, "p": adep})
                else:
                    fb = cast_group(gi)
                last_grp = gi == len(grps) - 1
                if not last_grp:
                    for k in range(k0, k1):
                        kk = k - k0
                        lhsT = lhsT_of(k)
                        for ci, (c0, cn) in enumerate(CHUNKS):
                            nc.tensor.matmul(
                                psum2[ci][:, 0:cn],
                                lhsT,
                                fb[:, kk * SP + c0: kk * SP + c0 + cn],
                                start=first, stop=False,
                            )
                        first = False
                else:
                    # final group: bank-outer with per-bank stops; ALL
                    # evictions are created after the matmuls (psum reads
                    # are tracked whole-tile, so an earlier-created read
                    # would falsely serialize the later banks' matmuls)
                    for ci, (c0, cn) in enumerate(CHUNKS):
                        for k in range(k0, k1):
                            kk = k - k0
                            nc.tensor.matmul(
                                psum2[ci][:, 0:cn],
                                lhsT_of(k),
                                fb[:, kk * SP + c0: kk * SP + c0 + cn],
                                start=False, stop=(k == k1 - 1),
                            )
                    # GPSIMD cannot read PSUM on HW: evict banks on DVE/ACT
                    dq_eng = (nc.sync, nc.scalar, nc.sync)
                    with tc.high_priority():
                        for ci, (c0, cn) in enumerate(CHUNKS):
                            if ci == 1:
                                nc.scalar.copy(
                                    ob[:, c0:c0 + cn], psum2[ci][:, 0:cn]
                                )
                            else:
                                nc.vector.tensor_copy(
                                    ob[:, c0:c0 + cn], psum2[ci][:, 0:cn]
                                )
                            dq_eng[ci].dma_start(
                                out[:, c0:c0 + cn], ob[:, c0:c0 + cn]
                            )

    nc.compile()
    return nc


def get_program(nocc=False, gsizes=GSIZES, ncast=NCAST, w1_ones=True,
                ndummy=NDUMMY):
    key = ("nc", nocc, tuple(gsizes), ncast, w1_ones, ndummy)
    if key not in _PROG_CACHE:
        _PROG_CACHE[key] = _build_program(nocc, gsizes, ncast, w1_ones,
                                          ndummy)
    return _PROG_CACHE[key]


def _slot_order():
    """Slot s = t*128 + p (tile t in 0..9, partition p) listed in quant-sort
    order: chunks of 4 (banks 0/1) or 2 (bank 2) consecutive sorted columns
    share one (chunk, partition) slot group, hence one dequant scale."""
    slots = np.empty(SP, np.int64)
    i = 0
    for ci, (tile0, ntile) in enumerate(((0, 4), (4, 4), (8, 2))):
        for p in range(128):
            for ti in range(ntile):
                slots[i] = (tile0 + ti) * 128 + p
                i += 1
    assert i == SP
    return slots


_SLOTS = _slot_order()


def _core_perm(colmax_ext):
    """perm[s] = original local dst column (or >=S for pad) in slot s, with
    columns sorted by quant range so slot groups share a scale."""
    order = np.argsort(-colmax_ext, kind="stable")  # [SP] sorted col ids
    perm = np.empty(SP, np.int64)
    perm[_SLOTS] = order
    return perm


def build_in_maps(x, src, dst, vals, W):
    """Host-side prep: dense A^T shard (u8 quantized, 4 sorted columns per
    scale group) + x0, both in the per-core permuted slot order."""
    import scipy.sparse as sp

    x = np.asarray(x, np.float32)
    src = np.asarray(src, np.int64)
    dst = np.asarray(dst, np.int64)
    vals = np.asarray(vals, np.float32)
    W = np.asarray(W, np.float32)

    # A[dst, src] = sum of vals  ->  we build AT[src, dst]
    AT = sp.coo_matrix((vals, (src, dst)), shape=(N, N)).toarray()

    # per-core column permutations (dst side of A, src rows of A, x rows)
    perms = []
    steps = []
    cscs = []
    for c in range(NCORES):
        ATc = AT[:, c * S:(c + 1) * S]  # [N, S] float32
        colmax_ext = np.full(SP, -1.0, np.float32)
        colmax_ext[:S] = ATc.max(axis=0)
        perm = _core_perm(colmax_ext)
        # group scale = max colmax over each slot group (same (chunk, p))
        cm_slot = np.maximum(colmax_ext[perm], 1e-9)  # [SP] by slot
        step_slot = np.empty(SP, np.float32)
        csc = np.empty((128, 3), np.float32)
        for ci, (tile0, ntile) in enumerate(((0, 4), (4, 4), (8, 2))):
            t_sl = slice(tile0 * 128, (tile0 + ntile) * 128)
            cm = cm_slot[t_sl].reshape(ntile, 128)    # [ntile, p]
            gmax = cm.max(axis=0) / 255.0             # [p]
            csc[:, ci] = gmax
            step_slot[t_sl] = np.tile(gmax[None, :], (ntile, 1)).reshape(-1)
        perms.append(perm)
        steps.append(step_slot)
        cscs.append(np.ascontiguousarray(csc))

    # per-core src slot -> node mapping: row-block i of core c is rank
    # (c+i)%8 (own rank first, so layer 2 starts from on-chip activations),
    # permuted within the block by that rank's own column permutation
    node2s, valid2s = [], []
    for c in range(NCORES):
        node2 = np.empty(NPAD, np.int64)
        valid2 = np.empty(NPAD, bool)
        for i in range(NCORES):
            r = (c + i) % NCORES
            pr = perms[r]
            valid = pr < S
            node2[i * SP:(i + 1) * SP] = np.where(valid, r * S + pr, 0)
            valid2[i * SP:(i + 1) * SP] = valid
        node2s.append(node2)
        valid2s.append(valid2)

    xw = x * W[0][None, :]

    w1brow = np.ascontiguousarray(
        np.tile(W[1][None, :], (128, 4))
    ).astype(np.float16)

    in_maps = []
    for c in range(NCORES):
        node2, valid2 = node2s[c], valid2s[c]
        x0p = np.zeros((NPAD, D), np.float32)
        x0p[valid2] = xw[node2[valid2]]
        x0h = np.ascontiguousarray(
            x0p.reshape(KT, 128, D).transpose(1, 0, 2).reshape(128, KT * D)
        ).astype(np.float16)
        ATc = AT[:, c * S:(c + 1) * S]  # [N, S] float32
        perm = perms[c]
        valid = perm < S
        ATs = np.zeros((N, SP), np.float32)
        ATs[:, valid] = ATc[:, perm[valid]]           # columns in slot order
        Aq = np.clip(np.rint(ATs / steps[c][None, :]), 0, 255).astype(
            np.uint8
        )
        Ap = np.zeros((NPAD, SP), Aq.dtype)
        Ap[valid2] = Aq[node2[valid2]]                # rows in slot order
        a3 = np.ascontiguousarray(Ap.reshape(KT, 128, SP))
        in_maps.append(
            {
                "a": a3,
                "x0": x0h,
                "csc": cscs[c],
                "w1b": w1brow,
            }
        )
    return in_maps, (steps, perms)


def assemble_output(results, aux):
    steps, perms = aux
    outs = []
    for c in range(NCORES):
        ot = np.asarray(results[c]["out"], np.float32)  # [128, SP] feat-major
        ot = ot * steps[c][None, :]  # per-dst dequant (layer-2)
        perm = perms[c]
        valid = perm < S
        o = np.zeros((S, 128), np.float32)
        o[perm[valid]] = ot[:, valid].T             # un-permute dst slots
        outs.append(o)
    return np.ascontiguousarray(np.concatenate(outs, axis=0))


def kernel(x, src, dst, vals, W):
    from concourse import bass_utils

    w1_ones = bool(np.all(np.asarray(W)[1] == 1.0))
    nc = get_program(w1_ones=w1_ones)
    in_maps, steps = build_in_maps(x, src, dst, vals, W)
    # The axon terminal can wedge when a different program was loaded
    # earlier in its lifetime; after the crash the terminal restarts and a
    # retry succeeds.  Back off progressively to ride out the restart.
    import time as _time

    last_err = None
    for sleep_s in (10.0, 30.0, 60.0, 0.0):
        try:
            res = bass_utils.run_bass_kernel_spmd(
                nc, in_maps, core_ids=list(range(NCORES))
            )
            return assemble_output(res.results, steps)
        except Exception as e:  # noqa: BLE001
            last_err = e
            _time.sleep(sleep_s)
    raise last_err


# revision 42
# speedup vs baseline: 1.4467x; 1.0044x over previous
"""GCN diag-encoder (2-layer SpMM) on 8 Trainium2 NeuronCores.

Strategy: the sparse adjacency (640K edges over 10K nodes, ~0.64% dense) is
materialized as a dense A^T matrix on the host; each per-layer
  out[dst] = sum_e vals[e] * x[src[e]]        (segment-sum SpMM)
becomes dense TensorEngine matmuls.  Each core owns a 1250-wide dst slice of
A^T (padded to 1280, uint8-quantized per dst column).

v3: A^T is DMA'd ONCE as raw uint8 (half the DMA bytes of a u8->f16
cast-DMA, which is charged at the f16 destination size) and stays resident
in SBUF (100KB/partition).  The u8->f16 conversion runs on-chip, split
across the three otherwise-idle compute engines (DVE / Activation / GpSimd)
into a rotating ring of f16 staging tiles that feed the PE.  Both layers
re-cast from the same resident u8 copy, so layer 2 needs no A traffic at
all.  This turns layer 1 from DMA-bound (~93us) into PE-bound (~45us) and
removes layer 2's 29us f16 re-stream.

Layer 1 runs A-stationary — matmul(out=psum[dst,feat], lhsT=AT_tile[src,dst],
rhs=x_tile[src,feat]) — so the layer-1 output is node-major: the eviction is
a fused tanh+dequant-scale pass on the scalar engine straight into the
AllGather bounce.  The host sorts each core's dst columns by quantization
range and packs 4 similar columns per (psum bank, partition) slot, so the
dequant scale is per-partition within a bank and the whole eviction is 3
bank-wide activations (full per-column accuracy at bank-chunk cost).  Each
psum bank is its own tile (psum reads are dependency-tracked whole-tile, so
per-bank tiles let each bank's eviction start at its own stop) and is
seeded by one full-width start=True zero matmul.  Layer 2 runs X-stationary
— matmul(out=psum[feat,dst], lhsT=x1_tile[src,feat], rhs=AT_tile[src,dst]);
its dequant scale and the final un-permute are applied on the host.

Src row-blocks are rotated per core so block 0 is the core's OWN rank:
layer 2's first 10 k-tiles read the tanh output agin_sb directly from SBUF
(no AllGather round-trip), hiding most of the collective latency behind
real work; the other 7 blocks are fetched from the AllGather output at
register-computed offsets ((partition_id+i)%8).  A few zl-by-zl keep-warm
matmuls stop the PE from dropping out of its max p-state across the
remaining gap.  W0 is folded into x on the host; W1 is skipped on device
when it is all-ones (torch init), else applied via a broadcast multiply.
"""

import numpy as np

N = 10000          # nodes
D = 128            # feature dim
NCORES = 8
S = 1250           # dst nodes per core
SP = 1280          # padded dst per core (10 tiles of 128)
KT = 80            # contraction k-tiles (padded src rows = 10240)
NPAD = KT * 128    # 10240
GSIZES = (1, 1, 2) + (4,) * 19   # k-tiles per group (sum = 80)
NCAST = 7          # f16 staging ring depth
NDUMMY = 6         # PE keep-warm matmuls bridging the AllGather valley
# psum bank chunks: layer-1 eviction + layer-2 column blocking
CHUNKS = ((0, 512), (512, 512), (1024, 256))

_PROG_CACHE = {}


def _build_program(nocc=False, gsizes=GSIZES, ncast=NCAST, w1_ones=True,
                   ndummy=NDUMMY):
    import concourse.bacc as bacc
    import concourse.mybir as mybir
    from bass_rust import InstructionNameOrderedSet as _NameSet
    from concourse import tile

    f32 = mybir.dt.float32
    f16 = mybir.dt.float16
    u8 = mybir.dt.uint8
    grps = []
    _k0 = 0
    for _sz in gsizes:
        grps.append((_k0, _k0 + _sz))
        _k0 += _sz
    assert _k0 == KT
    maxg = max(k1 - k0 for k0, k1 in grps)

    nc = bacc.Bacc(
        "TRN2",
        target_bir_lowering=False,
        debug=False,
        enable_asserts=False,
        num_devices=1 if nocc else NCORES,
    )

    a = nc.dram_tensor("a", [KT, 128, SP], u8, kind="ExternalInput").ap()
    x0 = nc.dram_tensor("x0", [128, NPAD], f16, kind="ExternalInput").ap()
    # per-(bank chunk, partition) dequant scales; the host sorts dst
    # columns by quant range so each (chunk, partition) slot's 4 columns
    # share one scale -> the tanh eviction is 3 bank-wide activations
    csc = nc.dram_tensor("csc", [128, 3], f32, kind="ExternalInput").ap()
    # broadcast W1 row tiled x4 (only read when not w1_ones)
    w1b = nc.dram_tensor("w1b", [128, 512], f16, kind="ExternalInput").ap()
    out = nc.dram_tensor("out", [128, SP], f32, kind="ExternalOutput").ap()

    with tile.TileContext(nc) as tc:
        with (
            tc.tile_pool(name="xp", bufs=1) as xp,
            tc.tile_pool(name="a8p", bufs=1) as a8p,
            tc.tile_pool(name="fc", bufs=ncast) as fcp,
            tc.tile_pool(name="ps", bufs=1, space="PSUM") as ps,
            tc.tile_pool(name="dr", bufs=1, space="DRAM") as dr,
        ):
            # x0 is dead once layer 1 finishes; share one slot for both
            x0s = xp.tile([128, NPAD], f16, tag="xs")
            x1s = xp.tile([128, NPAD], f16, tag="xs")
            cscs = xp.tile([128, 3], f32, tag="cscs")
            w1s = xp.tile([128, 512], f16, tag="w1s")
            zl = xp.tile([128, 512], f16, tag="zl")
            warm = xp.tile([128, 1], f32, tag="warm")
            nc.scalar.dma_start(cscs[:], csc)
            if not w1_ones:
                nc.scalar.dma_start(w1s[:], w1b)
            nc.vector.memset(zl[:], 0.0)
            # pre-load the ACT tanh table so the layer-1 eviction doesn't
            # pay the table load on the critical path
            nc.scalar.activation(
                warm[:], zl[:, 0:1], mybir.ActivationFunctionType.Tanh
            )

            agin = dr.tile([128, SP], f16)
            agout = dr.tile([NCORES * 128, SP], f16, addr_space="Shared")

            a8_tiles = {}

            def cast_group(gi, engines="vap", deps=None):
                """u8 -> f16 of resident group gi, split over the engines in
                `engines` (v=DVE, a=ACT in ~1us slices, p=POOL), shares
                proportional to their elementwise rates.  `deps` maps an
                engine letter to instruction names the slice must follow —
                used at the layer boundary so the scheduler cannot hoist
                casts ahead of the tanh -> AllGather chain."""
                k0, k1 = grps[gi]
                w = (k1 - k0) * SP
                a8 = a8_tiles[gi]
                fb = fcp.tile([128, maxg * SP], f16, tag="fc")
                rates = {"v": 4, "a": 5, "p": 3}
                tot = sum(rates[e] for e in engines)
                c0 = 0
                for e in engines:
                    c1 = w if e == engines[-1] else c0 + (w * rates[e]) // tot
                    insts = []
                    if e == "v":
                        insts.append(
                            nc.vector.tensor_copy(fb[:, c0:c1], a8[:, c0:c1])
                        )
                    elif e == "a":
                        # <=1.3k-elem slices so ACT never blocks the layer-1
                        # eviction chain behind a long copy
                        s0 = c0
                        while s0 < c1:
                            s1 = min(s0 + 1280, c1)
                            insts.append(
                                nc.scalar.copy(fb[:, s0:s1], a8[:, s0:s1])
                            )
                            s0 = s1
                    else:
                        insts.append(
                            nc.gpsimd.tensor_copy(fb[:, c0:c1], a8[:, c0:c1])
                        )
                    if deps and e in deps:
                        for inst in insts:
                            inst.ins.add_sync_dependencies_from(_NameSet(deps[e]))
                    c0 = c1
                return fb

            # ---- layer 1 (A-stationary; psum is [dst slot, feat]) ----
            # one psum tile per 2KiB bank: psum reads are dependency-tracked
            # whole-tile, so per-bank tiles let each bank's eviction start at
            # its own stop instead of after the layer's last matmul
            psum1 = []
            for ci, (c0, cn) in enumerate(CHUNKS):
                p1t = ps.tile([128, cn], f32, tag=f"acc1_{ci}", name=f"p1_{ci}")
                psum1.append(p1t)
            for ci, (c0, cn) in enumerate(CHUNKS):
                nc.tensor.matmul(
                    psum1[ci][:, 0:cn], zl[:, 0:128], zl[:, 0:cn],
                    start=True, stop=False,
                )
            # x0 for the first four groups rides ahead of their a8 loads so
            # the DMA queue can stay a couple of groups in front of the PE
            xlead = grps[3][1] * 128
            for gi, (k0, k1) in enumerate(grps):
                a8 = a8p.tile([128, (k1 - k0) * SP], u8, tag=f"a8_{gi}")
                a8_tiles[gi] = a8
                nc.sync.dma_start(
                    a8[:].rearrange("p (k j) -> p k j", k=k1 - k0),
                    a[k0:k1].rearrange("k p j -> p k j"),
                )
                if gi == 0:
                    nc.sync.dma_start(x0s[:, 0:xlead], x0[:, 0:xlead])
                if gi >= 4:
                    nc.sync.dma_start(
                        x0s[:, k0 * 128:k1 * 128], x0[:, k0 * 128:k1 * 128]
                    )
                fb = cast_group(gi)
                if gi < len(grps) - 1:
                    for k in range(k0, k1):
                        kk = k - k0
                        rhs = x0s[:, k * 128:(k + 1) * 128]
                        for t in range(10):
                            ci, tt = (t // 4, t % 4)
                            nc.tensor.matmul(
                                psum1[ci][:, tt * 128:(tt + 1) * 128],
                                fb[:, kk * SP + t * 128:
                                   kk * SP + (t + 1) * 128],
                                rhs,
                                start=False, stop=False,
                            )
                else:
                    # final group t-outer: each dst range finishes early so
                    # the tanh eviction overlaps the remaining matmuls
                    for t in range(10):
                        ci, tt = (t // 4, t % 4)
                        for k in range(k0, k1):
                            kk = k - k0
                            last_mm = nc.tensor.matmul(
                                psum1[ci][:, tt * 128:(tt + 1) * 128],
                                fb[:, kk * SP + t * 128:
                                   kk * SP + (t + 1) * 128],
                                x0s[:, k * 128:(k + 1) * 128],
                                start=False,
                                stop=(k == KT - 1 and t in (3, 7, 9)),
                            )

            # evict layer 1: x1 = tanh(cs_dst * psum1) [* W1] on ACT; DMA to
            # the AllGather bounce per psum bank so agin lands early.  The
            # whole tanh -> agin -> AllGather -> x1s chain is the only work
            # between the two PE-bound layers, so it runs at high priority
            # and its DMAs ride the otherwise-idle SP queue.
            agin_sb = xp.tile([128, SP], f16, tag="agin")
            # keep-warm matmuls: PE would otherwise idle across the AllGather
            # valley and restart cold (2.4x slower for the first 3us)
            psumd = ps.tile([128, 512], f32, tag="warmups")
            for _ in range(ndummy):
                dmm = nc.tensor.matmul(
                    psumd[:], zl[:, 0:128], zl[:, 0:512],
                    start=True, stop=True, skip_group_check=True,
                )
                # pin behind layer 1 so the scheduler cannot hoist the
                # warm-up matmuls to the (DMA-bound) start of the program
                dmm.ins.add_sync_dependencies_from(_NameSet([last_mm.ins.name]))
            tanh_last = None
            with tc.high_priority():
                for ci, (c0, cn) in enumerate(CHUNKS):
                    tanh_last = nc.scalar.activation(
                        agin_sb[:, c0:c0 + cn], psum1[ci][:, 0:cn],
                        mybir.ActivationFunctionType.Tanh,
                        scale=cscs[:, ci:ci + 1],
                    )
                    if not w1_ones:
                        nc.vector.tensor_mul(
                            agin_sb[:, c0:c0 + cn], agin_sb[:, c0:c0 + cn],
                            w1s[:, 0:cn]
                        )
                    nc.sync.dma_start(
                        agin[:, c0:c0 + cn], agin_sb[:, c0:c0 + cn]
                    )

                if nocc:
                    ag_inst = nc.sync.dma_start(agout[0:128, :], agin[:])
                else:
                    ag_inst = nc.gpsimd.collective_compute(
                        "AllGather",
                        mybir.AluOpType.bypass,
                        replica_groups=[list(range(NCORES))],
                        ins=[agin.opt()],
                        outs=[agout.opt()],
                    )
                # A's src row-blocks are rotated per core so block 0 is the
                # core's OWN rank: layer 2's first 10 k-tiles read agin_sb
                # directly (no AllGather round-trip), and block i (i>=1) is
                # rank (pid+i)%8, fetched from agout at a register-computed
                # offset.
                agdep = _NameSet([ag_inst.ins.name])
                pid = nc.sync.partition_id()
                x1dmas = []
                for i in range(1, NCORES):
                    boff = ((pid + i) % NCORES) * (128 * SP)
                    if i == 1:
                        # lead slice so k-tile 10 can start while the rest
                        # of the block is in flight
                        dsrc = agout[0:128, 0:384].copy()
                        dsrc.offset = boff
                        x1dmas.append(
                            nc.sync.dma_start(x1s[:, SP:SP + 384], dsrc)
                        )
                        dsrc = agout[0:128, 384:SP].copy()
                        dsrc.offset = boff + 384
                        x1dmas.append(
                            nc.sync.dma_start(
                                x1s[:, SP + 384:2 * SP], dsrc
                            )
                        )
                    else:
                        dsrc = agout[0:128, :].copy()
                        dsrc.offset = boff
                        x1dmas.append(
                            nc.sync.dma_start(
                                x1s[:, i * SP:(i + 1) * SP], dsrc
                            )
                        )
                # DRAM->SBUF reads of the collective output are not tracked
                # as data deps in the single-core twin; pin them so the
                # scheduler cannot float them ahead of the agin writes
                for d in x1dmas:
                    d.ins.add_sync_dependencies_from(agdep)

            # ---- layer 2 (X-stationary; psum is [feat, dst]) ----
            # All of A is already resident as u8; only the casts re-run.
            # The first ring of casts has no x1 dependency, so it completes
            # during the AllGather and PE starts as soon as rank 0 lands.
            psum2 = []
            for ci, (c0, cn) in enumerate(CHUNKS):
                p2t = ps.tile([128, cn], f32, tag=f"acc2_{ci}", name=f"p2_{ci}")
                psum2.append(p2t)
            ob = xp.tile([128, SP], f32, tag="ob")

            def lhsT_of(k):
                # row-block 0 is the core's own rank: its activations are
                # already on-chip in agin_sb (same [dst slot, feat] layout)
                if k < 10:
                    return agin_sb[:, k * 128:(k + 1) * 128]
                return x1s[:, k * 128:(k + 1) * 128]

            first = True
            tdep = [tanh_last.ins.name]
            adep = [ag_inst.ins.name]
            for gi, (k0, k1) in enumerate(grps):
                # keep ACT free for the tanh chain and POOL free for the
                # AllGather issue while the boundary groups pre-cast on DVE;
                # ACT/POOL rejoin once their part of the chain retires
                if gi < 3:
                    fb = cast_group(gi, "v")
                elif gi < 5:
                    fb = cast_group(gi, "va", deps={"a": tdep})
                elif gi < 9:
                    fb = cast_group(gi, "vap", deps={"a": tdep, "p": adep})
                else:
                    fb = cast_group(gi)
                last_grp = gi == len(grps) - 1
                if not last_grp:
                    for k in range(k0, k1):
                        kk = k - k0
                        lhsT = lhsT_of(k)
                        for ci, (c0, cn) in enumerate(CHUNKS):
                            nc.tensor.matmul(
                                psum2[ci][:, 0:cn],
                                lhsT,
                                fb[:, kk * SP + c0: kk * SP + c0 + cn],
                                start=first, stop=False,
                            )
                        first = False
                else:
                    # final group: bank-outer with per-bank stops; ALL
                    # evictions are created after the matmuls (psum reads
                    # are tracked whole-tile, so an earlier-created read
                    # would falsely serialize the later banks' matmuls)
                    for ci, (c0, cn) in enumerate(CHUNKS):
                        for k in range(k0, k1):
                            kk = k - k0
                            nc.tensor.matmul(
                                psum2[ci][:, 0:cn],
                                lhsT_of(k),
                                fb[:, kk * SP + c0: kk * SP + c0 + cn],
                                start=False, stop=(k == k1 - 1),
                            )
                    # GPSIMD cannot read PSUM on HW: evict banks on DVE/ACT
                    dq_eng = (nc.sync, nc.scalar, nc.sync)
                    with tc.high_priority():
                        for ci, (c0, cn) in enumerate(CHUNKS):
                            if ci == 1:
                                nc.scalar.copy(
                                    ob[:, c0:c0 + cn], psum2[ci][:, 0:cn]
                                )
                            else:
                                nc.vector.tensor_copy(
                                    ob[:, c0:c0 + cn], psum2[ci][:, 0:cn]
                                )
                            dq_eng[ci].dma_start(
                                out[:, c0:c0 + cn], ob[:, c0:c0 + cn]
                            )

    nc.compile()
    return nc


def get_program(nocc=False, gsizes=GSIZES, ncast=NCAST, w1_ones=True,
                ndummy=NDUMMY):
    key = ("nc", nocc, tuple(gsizes), ncast, w1_ones, ndummy)
    if key not in _PROG_CACHE:
        _PROG_CACHE[key] = _build_program(nocc, gsizes, ncast, w1_ones,
                                          ndummy)
    return _PROG_CACHE[key]


def _slot_order():
    """Slot s = t*128 + p (tile t in 0..9, partition p) listed in quant-sort
    order: chunks of 4 (banks 0/1) or 2 (bank 2) consecutive sorted columns
    share one (chunk, partition) slot group, hence one dequant scale."""
    slots = np.empty(SP, np.int64)
    i = 0
    for ci, (tile0, ntile) in enumerate(((0, 4), (4, 4), (8, 2))):
        for p in range(128):
            for ti in range(ntile):
                slots[i] = (tile0 + ti) * 128 + p
                i += 1
    assert i == SP
    return slots


_SLOTS = _slot_order()


def _core_perm(colmax_ext):
    """perm[s] = original local dst column (or >=S for pad) in slot s, with
    columns sorted by quant range so slot groups share a scale."""
    order = np.argsort(-colmax_ext, kind="stable")  # [SP] sorted col ids
    perm = np.empty(SP, np.int64)
    perm[_SLOTS] = order
    return perm


def build_in_maps(x, src, dst, vals, W):
    """Host-side prep: dense A^T shard (u8 quantized, 4 sorted columns per
    scale group) + x0, both in the per-core permuted slot order."""
    import scipy.sparse as sp

    x = np.asarray(x, np.float32)
    src = np.asarray(src, np.int64)
    dst = np.asarray(dst, np.int64)
    vals = np.asarray(vals, np.float32)
    W = np.asarray(W, np.float32)

    # A[dst, src] = sum of vals  ->  we build AT[src, dst]
    AT = sp.coo_matrix((vals, (src, dst)), shape=(N, N)).toarray()

    # per-core column permutations (dst side of A, src rows of A, x rows)
    perms = []
    steps = []
    cscs = []
    for c in range(NCORES):
        ATc = AT[:, c * S:(c + 1) * S]  # [N, S] float32
        colmax_ext = np.full(SP, -1.0, np.float32)
        colmax_ext[:S] = ATc.max(axis=0)
        perm = _core_perm(colmax_ext)
        # group scale = max colmax over each slot group (same (chunk, p))
        cm_slot = np.maximum(colmax_ext[perm], 1e-9)  # [SP] by slot
        step_slot = np.empty(SP, np.float32)
        csc = np.empty((128, 3), np.float32)
        for ci, (tile0, ntile) in enumerate(((0, 4), (4, 4), (8, 2))):
            t_sl = slice(tile0 * 128, (tile0 + ntile) * 128)
            cm = cm_slot[t_sl].reshape(ntile, 128)    # [ntile, p]
            gmax = cm.max(axis=0) / 255.0             # [p]
            csc[:, ci] = gmax
            step_slot[t_sl] = np.tile(gmax[None, :], (ntile, 1)).reshape(-1)
        perms.append(perm)
        steps.append(step_slot)
        cscs.append(np.ascontiguousarray(csc))

    # per-core src slot -> node mapping: row-block i of core c is rank
    # (c+i)%8 (own rank first, so layer 2 starts from on-chip activations),
    # permuted within the block by that rank's own column permutation
    node2s, valid2s = [], []
    for c in range(NCORES):
        node2 = np.empty(NPAD, np.int64)
        valid2 = np.empty(NPAD, bool)
        for i in range(NCORES):
            r = (c + i) % NCORES
            pr = perms[r]
            valid = pr < S
            node2[i * SP:(i + 1) * SP] = np.where(valid, r * S + pr, 0)
            valid2[i * SP:(i + 1) * SP] = valid
        node2s.append(node2)
        valid2s.append(valid2)

    xw = x * W[0][None, :]

    w1brow = np.ascontiguousarray(
        np.tile(W[1][None, :], (128, 4))
    ).astype(np.float16)

    in_maps = []
    for c in range(NCORES):
        node2, valid2 = node2s[c], valid2s[c]
        x0p = np.zeros((NPAD, D), np.float32)
        x0p[valid2] = xw[node2[valid2]]
        x0h = np.ascontiguousarray(
            x0p.reshape(KT, 128, D).transpose(1, 0, 2).reshape(128, KT * D)
        ).astype(np.float16)
        ATc = AT[:, c * S:(c + 1) * S]  # [N, S] float32
        perm = perms[c]
        valid = perm < S
        ATs = np.zeros((N, SP), np.float32)
        ATs[:, valid] = ATc[:, perm[valid]]           # columns in slot order
        Aq = np.clip(np.rint(ATs / steps[c][None, :]), 0, 255).astype(
            np.uint8
        )
        Ap = np.zeros((NPAD, SP), Aq.dtype)
        Ap[valid2] = Aq[node2[valid2]]                # rows in slot order
        a3 = np.ascontiguousarray(Ap.reshape(KT, 128, SP))
        in_maps.append(
            {
                "a": a3,
                "x0": x0h,
                "csc": cscs[c],
                "w1b": w1brow,
            }
        )
    return in_maps, (steps, perms)


def assemble_output(results, aux):
    steps, perms = aux
    outs = []
    for c in range(NCORES):
        ot = np.asarray(results[c]["out"], np.float32)  # [128, SP] feat-major
        ot = ot * steps[c][None, :]  # per-dst dequant (layer-2)
        perm = perms[c]
        valid = perm < S
        o = np.zeros((S, 128), np.float32)
        o[perm[valid]] = ot[:, valid].T             # un-permute dst slots
        outs.append(o)
    return np.ascontiguousarray(np.concatenate(outs, axis=0))


def kernel(x, src, dst, vals, W):
    from concourse import bass_utils

    w1_ones = bool(np.all(np.asarray(W)[1] == 1.0))
    nc = get_program(w1_ones=w1_ones)
    in_maps, steps = build_in_maps(x, src, dst, vals, W)
    # The axon terminal can wedge when a different program was loaded
    # earlier in its lifetime; after the crash the terminal restarts and a
    # retry succeeds.  Back off progressively to ride out the restart.
    import time as _time

    last_err = None
    for sleep_s in (10.0, 30.0, 60.0, 0.0):
        try:
            res = bass_utils.run_bass_kernel_spmd(
                nc, in_maps, core_ids=list(range(NCORES))
            )
            return assemble_output(res.results, steps)
        except Exception as e:  # noqa: BLE001
            last_err = e
            _time.sleep(sleep_s)
    raise last_err


# revision 48
# speedup vs baseline: 1.4536x; 1.0048x over previous
"""GCN diag-encoder (2-layer SpMM) on 8 Trainium2 NeuronCores.

Strategy: the sparse adjacency (640K edges over 10K nodes, ~0.64% dense) is
materialized as a dense A^T matrix on the host; each per-layer
  out[dst] = sum_e vals[e] * x[src[e]]        (segment-sum SpMM)
becomes dense TensorEngine matmuls.  Each core owns a 1250-wide dst slice of
A^T (padded to 1280, uint8-quantized per dst column).

v3: A^T is DMA'd ONCE as raw uint8 (half the DMA bytes of a u8->f16
cast-DMA, which is charged at the f16 destination size) and stays resident
in SBUF (100KB/partition).  The u8->f16 conversion runs on-chip, split
across the three otherwise-idle compute engines (DVE / Activation / GpSimd)
into a rotating ring of f16 staging tiles that feed the PE.  Both layers
re-cast from the same resident u8 copy, so layer 2 needs no A traffic at
all.  This turns layer 1 from DMA-bound (~93us) into PE-bound (~45us) and
removes layer 2's 29us f16 re-stream.

Layer 1 runs A-stationary — matmul(out=psum[dst,feat], lhsT=AT_tile[src,dst],
rhs=x_tile[src,feat]) — so the layer-1 output is node-major: the eviction is
a fused tanh+dequant-scale pass on the scalar engine straight into the
AllGather bounce.  The host sorts each core's dst columns by quantization
range and packs 4 similar columns per (psum bank, partition) slot, so the
dequant scale is per-partition within a bank and the whole eviction is 3
bank-wide activations (full per-column accuracy at bank-chunk cost).  Each
psum bank is its own tile (psum reads are dependency-tracked whole-tile, so
per-bank tiles let each bank's eviction start at its own stop) and is
seeded by one full-width start=True zero matmul.  Layer 2 runs X-stationary
— matmul(out=psum[feat,dst], lhsT=x1_tile[src,feat], rhs=AT_tile[src,dst]);
its dequant scale and the final un-permute are applied on the host.

Src row-blocks are rotated per core so block 0 is the core's OWN rank:
layer 2's first 10 k-tiles read the tanh output agin_sb directly from SBUF
(no AllGather round-trip), hiding most of the collective latency behind
real work; the other 7 blocks are fetched from the AllGather output at
register-computed offsets ((partition_id+i)%8).  A few zl-by-zl keep-warm
matmuls stop the PE from dropping out of its max p-state across the
remaining gap.  W0 is folded into x on the host; W1 is skipped on device
when it is all-ones (torch init), else applied via a broadcast multiply.
"""

import numpy as np

N = 10000          # nodes
D = 128            # feature dim
NCORES = 8
S = 1250           # dst nodes per core
SP = 1280          # padded dst per core (10 tiles of 128)
KT = 80            # contraction k-tiles (padded src rows = 10240)
NPAD = KT * 128    # 10240
GSIZES = (2, 2) + (4,) * 19      # k-tiles per group (sum = 80)
NCAST = 7          # f16 staging ring depth
NDUMMY = 6         # PE keep-warm matmuls bridging the AllGather valley
# psum bank chunks: layer-1 eviction + layer-2 column blocking
CHUNKS = ((0, 512), (512, 512), (1024, 256))

_PROG_CACHE = {}


def _build_program(nocc=False, gsizes=GSIZES, ncast=NCAST, w1_ones=True,
                   ndummy=NDUMMY):
    import concourse.bacc as bacc
    import concourse.mybir as mybir
    from bass_rust import InstructionNameOrderedSet as _NameSet
    from concourse import tile

    f32 = mybir.dt.float32
    f16 = mybir.dt.float16
    u8 = mybir.dt.uint8
    grps = []
    _k0 = 0
    for _sz in gsizes:
        grps.append((_k0, _k0 + _sz))
        _k0 += _sz
    assert _k0 == KT
    maxg = max(k1 - k0 for k0, k1 in grps)

    nc = bacc.Bacc(
        "TRN2",
        target_bir_lowering=False,
        debug=False,
        enable_asserts=False,
        num_devices=1 if nocc else NCORES,
    )

    a = nc.dram_tensor("a", [KT, 128, SP], u8, kind="ExternalInput").ap()
    x0 = nc.dram_tensor("x0", [128, NPAD], f16, kind="ExternalInput").ap()
    # per-(bank chunk, partition) dequant scales; the host sorts dst
    # columns by quant range so each (chunk, partition) slot's 4 columns
    # share one scale -> the tanh eviction is 3 bank-wide activations
    csc = nc.dram_tensor("csc", [128, 3], f32, kind="ExternalInput").ap()
    # broadcast W1 row tiled x4 (only read when not w1_ones)
    w1b = nc.dram_tensor("w1b", [128, 512], f16, kind="ExternalInput").ap()
    out = nc.dram_tensor("out", [128, SP], f32, kind="ExternalOutput").ap()

    with tile.TileContext(nc) as tc:
        with (
            tc.tile_pool(name="xp", bufs=1) as xp,
            tc.tile_pool(name="a8p", bufs=1) as a8p,
            tc.tile_pool(name="fc", bufs=ncast) as fcp,
            tc.tile_pool(name="ps", bufs=1, space="PSUM") as ps,
            tc.tile_pool(name="dr", bufs=1, space="DRAM") as dr,
        ):
            # x0 is dead once layer 1 finishes; share one slot for both
            x0s = xp.tile([128, NPAD], f16, tag="xs")
            x1s = xp.tile([128, NPAD], f16, tag="xs")
            cscs = xp.tile([128, 3], f32, tag="cscs")
            w1s = xp.tile([128, 512], f16, tag="w1s")
            zl = xp.tile([128, 512], f16, tag="zl")
            warm = xp.tile([128, 1], f32, tag="warm")
            nc.scalar.dma_start(cscs[:], csc)
            if not w1_ones:
                nc.scalar.dma_start(w1s[:], w1b)
            nc.vector.memset(zl[:], 0.0)
            # pre-load the ACT tanh table so the layer-1 eviction doesn't
            # pay the table load on the critical path
            nc.scalar.activation(
                warm[:], zl[:, 0:1], mybir.ActivationFunctionType.Tanh
            )

            agin = dr.tile([128, SP], f16)
            agout = dr.tile([NCORES * 128, SP], f16, addr_space="Shared")

            a8_tiles = {}

            def cast_group(gi, engines="vap", deps=None):
                """u8 -> f16 of resident group gi, split over the engines in
                `engines` (v=DVE, a=ACT in ~1us slices, p=POOL), shares
                proportional to their elementwise rates.  `deps` maps an
                engine letter to instruction names the slice must follow —
                used at the layer boundary so the scheduler cannot hoist
                casts ahead of the tanh -> AllGather chain."""
                k0, k1 = grps[gi]
                w = (k1 - k0) * SP
                a8 = a8_tiles[gi]
                fb = fcp.tile([128, maxg * SP], f16, tag="fc")
                rates = {"v": 4, "a": 5, "p": 3}
                tot = sum(rates[e] for e in engines)
                halves = deps.pop("halves", 1) if deps else 1
                bounds = [(w * h) // halves for h in range(halves + 1)]
                for h0, h1 in zip(bounds, bounds[1:]):
                  wh = h1 - h0
                  c0 = h0
                  for e in engines:
                    c1 = h1 if e == engines[-1] else c0 + (wh * rates[e]) // tot
                    insts = []
                    if e == "v":
                        insts.append(
                            nc.vector.tensor_copy(fb[:, c0:c1], a8[:, c0:c1])
                        )
                    elif e == "a":
                        # <=1.3k-elem slices so ACT never blocks the layer-1
                        # eviction chain behind a long copy
                        s0 = c0
                        while s0 < c1:
                            s1 = min(s0 + 1280, c1)
                            insts.append(
                                nc.scalar.copy(fb[:, s0:s1], a8[:, s0:s1])
                            )
                            s0 = s1
                    else:
                        insts.append(
                            nc.gpsimd.tensor_copy(fb[:, c0:c1], a8[:, c0:c1])
                        )
                    if deps and e in deps:
                        for inst in insts:
                            inst.ins.add_sync_dependencies_from(_NameSet(deps[e]))
                    c0 = c1
                return fb

            # ---- layer 1 (A-stationary; psum is [dst slot, feat]) ----
            # one psum tile per 2KiB bank: psum reads are dependency-tracked
            # whole-tile, so per-bank tiles let each bank's eviction start at
            # its own stop instead of after the layer's last matmul
            psum1 = []
            for ci, (c0, cn) in enumerate(CHUNKS):
                p1t = ps.tile([128, cn], f32, tag=f"acc1_{ci}", name=f"p1_{ci}")
                psum1.append(p1t)
            for ci, (c0, cn) in enumerate(CHUNKS):
                nc.tensor.matmul(
                    psum1[ci][:, 0:cn], zl[:, 0:128], zl[:, 0:cn],
                    start=True, stop=False,
                )
            # x0 for the first four groups rides ahead of their a8 loads so
            # the DMA queue can stay a couple of groups in front of the PE
            xlead = grps[3][1] * 128
            for gi, (k0, k1) in enumerate(grps):
                a8 = a8p.tile([128, (k1 - k0) * SP], u8, tag=f"a8_{gi}")
                a8_tiles[gi] = a8
                nh = 2 if (3 <= gi <= 6 and (k1 - k0) % 2 == 0) else 1
                kb = [k0 + ((k1 - k0) * h) // nh for h in range(nh + 1)]
                for b0, b1 in zip(kb, kb[1:]):
                    nc.sync.dma_start(
                        a8[:, (b0 - k0) * SP:(b1 - k0) * SP].rearrange(
                            "p (k j) -> p k j", k=b1 - b0
                        ),
                        a[b0:b1].rearrange("k p j -> p k j"),
                    )
                if gi == 0:
                    nc.sync.dma_start(x0s[:, 0:xlead], x0[:, 0:xlead])
                if gi >= 4:
                    nc.sync.dma_start(
                        x0s[:, k0 * 128:k1 * 128], x0[:, k0 * 128:k1 * 128]
                    )
            ng = len(grps)
            for oi, gi in enumerate(range(ng)):
                k0, k1 = grps[gi]
                fb = cast_group(gi, deps={"halves": 2}
                                if 3 <= gi <= 6 else None)
                if oi < ng - 1:
                    for k in range(k0, k1):
                        kk = k - k0
                        rhs = x0s[:, k * 128:(k + 1) * 128]
                        for t in range(10):
                            ci, tt = (t // 4, t % 4)
                            nc.tensor.matmul(
                                psum1[ci][:, tt * 128:(tt + 1) * 128],
                                fb[:, kk * SP + t * 128:
                                   kk * SP + (t + 1) * 128],
                                rhs,
                                start=False, stop=False,
                            )
                else:
                    # final group t-outer: each dst range finishes early so
                    # the tanh eviction overlaps the remaining matmuls
                    for t in range(10):
                        ci, tt = (t // 4, t % 4)
                        for k in range(k0, k1):
                            kk = k - k0
                            last_mm = nc.tensor.matmul(
                                psum1[ci][:, tt * 128:(tt + 1) * 128],
                                fb[:, kk * SP + t * 128:
                                   kk * SP + (t + 1) * 128],
                                x0s[:, k * 128:(k + 1) * 128],
                                start=False,
                                stop=(k == k1 - 1 and t in (3, 7, 9)),
                            )

            # evict layer 1: x1 = tanh(cs_dst * psum1) [* W1] on ACT; DMA to
            # the AllGather bounce per psum bank so agin lands early.  The
            # whole tanh -> agin -> AllGather -> x1s chain is the only work
            # between the two PE-bound layers, so it runs at high priority
            # and its DMAs ride the otherwise-idle SP queue.
            agin_sb = xp.tile([128, SP], f16, tag="agin")
            # keep-warm matmuls: PE would otherwise idle across the AllGather
            # valley and restart cold (2.4x slower for the first 3us)
            psumd = ps.tile([128, 512], f32, tag="warmups")
            for _ in range(ndummy):
                dmm = nc.tensor.matmul(
                    psumd[:], zl[:, 0:128], zl[:, 0:512],
                    start=True, stop=True, skip_group_check=True,
                )
                # pin behind layer 1 so the scheduler cannot hoist the
                # warm-up matmuls to the (DMA-bound) start of the program
                dmm.ins.add_sync_dependencies_from(_NameSet([last_mm.ins.name]))
            tanh_last = None
            with tc.high_priority():
                for ci, (c0, cn) in enumerate(CHUNKS):
                    tanh_last = nc.scalar.activation(
                        agin_sb[:, c0:c0 + cn], psum1[ci][:, 0:cn],
                        mybir.ActivationFunctionType.Tanh,
                        scale=cscs[:, ci:ci + 1],
                    )
                    if not w1_ones:
                        nc.vector.tensor_mul(
                            agin_sb[:, c0:c0 + cn], agin_sb[:, c0:c0 + cn],
                            w1s[:, 0:cn]
                        )
                    nc.sync.dma_start(
                        agin[:, c0:c0 + cn], agin_sb[:, c0:c0 + cn]
                    )

                if nocc:
                    ag_inst = nc.sync.dma_start(agout[0:128, :], agin[:])
                else:
                    ag_inst = nc.gpsimd.collective_compute(
                        "AllGather",
                        mybir.AluOpType.bypass,
                        replica_groups=[list(range(NCORES))],
                        ins=[agin.opt()],
                        outs=[agout.opt()],
                    )
                # A's src row-blocks are rotated per core so block 0 is the
                # core's OWN rank: layer 2's first 10 k-tiles read agin_sb
                # directly (no AllGather round-trip), and block i (i>=1) is
                # rank (pid+i)%8, fetched from agout at a register-computed
                # offset.
                agdep = _NameSet([ag_inst.ins.name])
                pid = nc.sync.partition_id()
                x1dmas = []
                for i in range(1, NCORES):
                    boff = ((pid + i) % NCORES) * (128 * SP)
                    if i == 1:
                        # lead slice so k-tile 10 can start while the rest
                        # of the block is in flight
                        dsrc = agout[0:128, 0:384].copy()
                        dsrc.offset = boff
                        x1dmas.append(
                            nc.sync.dma_start(x1s[:, SP:SP + 384], dsrc)
                        )
                        dsrc = agout[0:128, 384:SP].copy()
                        dsrc.offset = boff + 384
                        x1dmas.append(
                            nc.sync.dma_start(
                                x1s[:, SP + 384:2 * SP], dsrc
                            )
                        )
                    else:
                        dsrc = agout[0:128, :].copy()
                        dsrc.offset = boff
                        x1dmas.append(
                            nc.sync.dma_start(
                                x1s[:, i * SP:(i + 1) * SP], dsrc
                            )
                        )
                # DRAM->SBUF reads of the collective output are not tracked
                # as data deps in the single-core twin; pin them so the
                # scheduler cannot float them ahead of the agin writes
                for d in x1dmas:
                    d.ins.add_sync_dependencies_from(agdep)

            # ---- layer 2 (X-stationary; psum is [feat, dst]) ----
            # All of A is already resident as u8; only the casts re-run.
            # The first ring of casts has no x1 dependency, so it completes
            # during the AllGather and PE starts as soon as rank 0 lands.
            psum2 = []
            for ci, (c0, cn) in enumerate(CHUNKS):
                p2t = ps.tile([128, cn], f32, tag=f"acc2_{ci}", name=f"p2_{ci}")
                psum2.append(p2t)
            ob = xp.tile([128, SP], f32, tag="ob")

            def lhsT_of(k):
                # row-block 0 is the core's own rank: its activations are
                # already on-chip in agin_sb (same [dst slot, feat] layout)
                if k < 10:
                    return agin_sb[:, k * 128:(k + 1) * 128]
                return x1s[:, k * 128:(k + 1) * 128]

            first = True
            tdep = [tanh_last.ins.name]
            adep = [ag_inst.ins.name]
            for gi, (k0, k1) in enumerate(grps):
                # keep ACT free for the tanh chain and POOL free for the
                # AllGather issue while the boundary groups pre-cast on DVE;
                # ACT/POOL rejoin once their part of the chain retires
                if gi < 3:
                    fb = cast_group(gi, "v")
                elif gi < 5:
                    fb = cast_group(gi, "va", deps={"a": tdep})
                elif gi < 9:
                    fb = cast_group(gi, "vap", deps={"a": tdep, "p": adep})
                else:
                    fb = cast_group(gi)
                last_grp = gi == len(grps) - 1
                if not last_grp:
                    for k in range(k0, k1):
                        kk = k - k0
                        lhsT = lhsT_of(k)
                        for ci, (c0, cn) in enumerate(CHUNKS):
                            nc.tensor.matmul(
                                psum2[ci][:, 0:cn],
                                lhsT,
                                fb[:, kk * SP + c0: kk * SP + c0 + cn],
                                start=first, stop=False,
                            )
                        first = False
                else:
                    # final group: bank-outer with per-bank stops; ALL
                    # evictions are created after the matmuls (psum reads
                    # are tracked whole-tile, so an earlier-created read
                    # would falsely serialize the later banks' matmuls)
                    for ci, (c0, cn) in enumerate(CHUNKS):
                        for k in range(k0, k1):
                            kk = k - k0
                            nc.tensor.matmul(
                                psum2[ci][:, 0:cn],
                                lhsT_of(k),
                                fb[:, kk * SP + c0: kk * SP + c0 + cn],
                                start=False, stop=(k == k1 - 1),
                            )
                    # GPSIMD cannot read PSUM on HW: evict banks on DVE/ACT
                    dq_eng = (nc.sync, nc.scalar, nc.sync)
                    with tc.high_priority():
                        for ci, (c0, cn) in enumerate(CHUNKS):
                            if ci == 1:
                                nc.scalar.copy(
                                    ob[:, c0:c0 + cn], psum2[ci][:, 0:cn]
                                )
                            else:
                                nc.vector.tensor_copy(
                                    ob[:, c0:c0 + cn], psum2[ci][:, 0:cn]
                                )
                            dq_eng[ci].dma_start(
                                out[:, c0:c0 + cn], ob[:, c0:c0 + cn]
                            )

    nc.compile()
    return nc


def get_program(nocc=False, gsizes=GSIZES, ncast=NCAST, w1_ones=True,
                ndummy=NDUMMY):
    key = ("nc", nocc, tuple(gsizes), ncast, w1_ones, ndummy)
    if key not in _PROG_CACHE:
        _PROG_CACHE[key] = _build_program(nocc, gsizes, ncast, w1_ones,
                                          ndummy)
    return _PROG_CACHE[key]


def _slot_order():
    """Slot s = t*128 + p (tile t in 0..9, partition p) listed in quant-sort
    order: chunks of 4 (banks 0/1) or 2 (bank 2) consecutive sorted columns
    share one (chunk, partition) slot group, hence one dequant scale."""
    slots = np.empty(SP, np.int64)
    i = 0
    for ci, (tile0, ntile) in enumerate(((0, 4), (4, 4), (8, 2))):
        for p in range(128):
            for ti in range(ntile):
                slots[i] = (tile0 + ti) * 128 + p
                i += 1
    assert i == SP
    return slots


_SLOTS = _slot_order()


def _core_perm(colmax_ext):
    """perm[s] = original local dst column (or >=S for pad) in slot s, with
    columns sorted by quant range so slot groups share a scale."""
    order = np.argsort(-colmax_ext, kind="stable")  # [SP] sorted col ids
    perm = np.empty(SP, np.int64)
    perm[_SLOTS] = order
    return perm


def build_in_maps(x, src, dst, vals, W):
    """Host-side prep: dense A^T shard (u8 quantized, 4 sorted columns per
    scale group) + x0, both in the per-core permuted slot order."""
    import scipy.sparse as sp

    x = np.asarray(x, np.float32)
    src = np.asarray(src, np.int64)
    dst = np.asarray(dst, np.int64)
    vals = np.asarray(vals, np.float32)
    W = np.asarray(W, np.float32)

    # A[dst, src] = sum of vals  ->  we build AT[src, dst]
    AT = sp.coo_matrix((vals, (src, dst)), shape=(N, N)).toarray()

    # per-core column permutations (dst side of A, src rows of A, x rows)
    perms = []
    steps = []
    cscs = []
    for c in range(NCORES):
        ATc = AT[:, c * S:(c + 1) * S]  # [N, S] float32
        colmax_ext = np.full(SP, -1.0, np.float32)
        colmax_ext[:S] = ATc.max(axis=0)
        perm = _core_perm(colmax_ext)
        # group scale = max colmax over each slot group (same (chunk, p))
        cm_slot = np.maximum(colmax_ext[perm], 1e-9)  # [SP] by slot
        step_slot = np.empty(SP, np.float32)
        csc = np.empty((128, 3), np.float32)
        for ci, (tile0, ntile) in enumerate(((0, 4), (4, 4), (8, 2))):
            t_sl = slice(tile0 * 128, (tile0 + ntile) * 128)
            cm = cm_slot[t_sl].reshape(ntile, 128)    # [ntile, p]
            gmax = cm.max(axis=0) / 255.0             # [p]
            csc[:, ci] = gmax
            step_slot[t_sl] = np.tile(gmax[None, :], (ntile, 1)).reshape(-1)
        perms.append(perm)
        steps.append(step_slot)
        cscs.append(np.ascontiguousarray(csc))

    # per-core src slot -> node mapping: row-block i of core c is rank
    # (c+i)%8 (own rank first, so layer 2 starts from on-chip activations),
    # permuted within the block by that rank's own column permutation
    node2s, valid2s = [], []
    for c in range(NCORES):
        node2 = np.empty(NPAD, np.int64)
        valid2 = np.empty(NPAD, bool)
        for i in range(NCORES):
            r = (c + i) % NCORES
            pr = perms[r]
            valid = pr < S
            node2[i * SP:(i + 1) * SP] = np.where(valid, r * S + pr, 0)
            valid2[i * SP:(i + 1) * SP] = valid
        node2s.append(node2)
        valid2s.append(valid2)

    xw = x * W[0][None, :]

    w1brow = np.ascontiguousarray(
        np.tile(W[1][None, :], (128, 4))
    ).astype(np.float16)

    in_maps = []
    for c in range(NCORES):
        node2, valid2 = node2s[c], valid2s[c]
        x0p = np.zeros((NPAD, D), np.float32)
        x0p[valid2] = xw[node2[valid2]]
        x0h = np.ascontiguousarray(
            x0p.reshape(KT, 128, D).transpose(1, 0, 2).reshape(128, KT * D)
        ).astype(np.float16)
        ATc = AT[:, c * S:(c + 1) * S]  # [N, S] float32
        perm = perms[c]
        valid = perm < S
        ATs = np.zeros((N, SP), np.float32)
        ATs[:, valid] = ATc[:, perm[valid]]           # columns in slot order
        Aq = np.clip(np.rint(ATs / steps[c][None, :]), 0, 255).astype(
            np.uint8
        )
        Ap = np.zeros((NPAD, SP), Aq.dtype)
        Ap[valid2] = Aq[node2[valid2]]                # rows in slot order
        a3 = np.ascontiguousarray(Ap.reshape(KT, 128, SP))
        in_maps.append(
            {
                "a": a3,
                "x0": x0h,
                "csc": cscs[c],
                "w1b": w1brow,
            }
        )
    return in_maps, (steps, perms)


def assemble_output(results, aux):
    steps, perms = aux
    outs = []
    for c in range(NCORES):
        ot = np.asarray(results[c]["out"], np.float32)  # [128, SP] feat-major
        ot = ot * steps[c][None, :]  # per-dst dequant (layer-2)
        perm = perms[c]
        valid = perm < S
        o = np.zeros((S, 128), np.float32)
        o[perm[valid]] = ot[:, valid].T             # un-permute dst slots
        outs.append(o)
    return np.ascontiguousarray(np.concatenate(outs, axis=0))


def kernel(x, src, dst, vals, W):
    from concourse import bass_utils

    w1_ones = bool(np.all(np.asarray(W)[1] == 1.0))
    nc = get_program(w1_ones=w1_ones)
    in_maps, steps = build_in_maps(x, src, dst, vals, W)
    # The axon terminal can wedge when a different program was loaded
    # earlier in its lifetime; after the crash the terminal restarts and a
    # retry succeeds.  Back off progressively to ride out the restart.
    import time as _time

    last_err = None
    for sleep_s in (10.0, 30.0, 60.0, 0.0):
        try:
            res = bass_utils.run_bass_kernel_spmd(
                nc, in_maps, core_ids=list(range(NCORES))
            )
            return assemble_output(res.results, steps)
        except Exception as e:  # noqa: BLE001
            last_err = e
            _time.sleep(sleep_s)
    raise last_err


# revision 49
# speedup vs baseline: 1.4575x; 1.0027x over previous
"""GCN diag-encoder (2-layer SpMM) on 8 Trainium2 NeuronCores.

Strategy: the sparse adjacency (640K edges over 10K nodes, ~0.64% dense) is
materialized as a dense A^T matrix on the host; each per-layer
  out[dst] = sum_e vals[e] * x[src[e]]        (segment-sum SpMM)
becomes dense TensorEngine matmuls.  Each core owns a 1250-wide dst slice of
A^T (padded to 1280, uint8-quantized per dst column).

v3: A^T is DMA'd ONCE as raw uint8 (half the DMA bytes of a u8->f16
cast-DMA, which is charged at the f16 destination size) and stays resident
in SBUF (100KB/partition).  The u8->f16 conversion runs on-chip, split
across the three otherwise-idle compute engines (DVE / Activation / GpSimd)
into a rotating ring of f16 staging tiles that feed the PE.  Both layers
re-cast from the same resident u8 copy, so layer 2 needs no A traffic at
all.  This turns layer 1 from DMA-bound (~93us) into PE-bound (~45us) and
removes layer 2's 29us f16 re-stream.

Layer 1 runs A-stationary — matmul(out=psum[dst,feat], lhsT=AT_tile[src,dst],
rhs=x_tile[src,feat]) — so the layer-1 output is node-major: the eviction is
a fused tanh+dequant-scale pass on the scalar engine straight into the
AllGather bounce.  The host sorts each core's dst columns by quantization
range and packs 4 similar columns per (psum bank, partition) slot, so the
dequant scale is per-partition within a bank and the whole eviction is 3
bank-wide activations (full per-column accuracy at bank-chunk cost).  Each
psum bank is its own tile (psum reads are dependency-tracked whole-tile, so
per-bank tiles let each bank's eviction start at its own stop) and is
seeded by one full-width start=True zero matmul.  Layer 2 runs X-stationary
— matmul(out=psum[feat,dst], lhsT=x1_tile[src,feat], rhs=AT_tile[src,dst]);
its dequant scale and the final un-permute are applied on the host.

Src row-blocks are rotated per core so block 0 is the core's OWN rank:
layer 2's first 10 k-tiles read the tanh output agin_sb directly from SBUF
(no AllGather round-trip), hiding most of the collective latency behind
real work; the other 7 blocks are fetched from the AllGather output at
register-computed offsets ((partition_id+i)%8).  A few zl-by-zl keep-warm
matmuls stop the PE from dropping out of its max p-state across the
remaining gap.  W0 is folded into x on the host; W1 is skipped on device
when it is all-ones (torch init), else applied via a broadcast multiply.
"""

import numpy as np

N = 10000          # nodes
D = 128            # feature dim
NCORES = 8
S = 1250           # dst nodes per core
SP = 1280          # padded dst per core (10 tiles of 128)
KT = 80            # contraction k-tiles (padded src rows = 10240)
NPAD = KT * 128    # 10240
GSIZES = (2, 2) + (4,) * 19      # k-tiles per group (sum = 80)
NCAST = 7          # f16 staging ring depth
NDUMMY = 6         # PE keep-warm matmuls bridging the AllGather valley
# psum bank chunks: layer-1 eviction + layer-2 column blocking
CHUNKS = ((0, 512), (512, 512), (1024, 256))
# the 30 pad columns per core sort to partitions 113..127 of tiles 8 and 9,
# so k-tiles with k%10 in (8,9) only have 113 real src rows
NPADROW = 113


def _rows(k):
    return NPADROW if k % 10 in (8, 9) else 128

_PROG_CACHE = {}


def _build_program(nocc=False, gsizes=GSIZES, ncast=NCAST, w1_ones=True,
                   ndummy=NDUMMY):
    import concourse.bacc as bacc
    import concourse.mybir as mybir
    from bass_rust import InstructionNameOrderedSet as _NameSet
    from concourse import tile

    f32 = mybir.dt.float32
    f16 = mybir.dt.float16
    u8 = mybir.dt.uint8
    grps = []
    _k0 = 0
    for _sz in gsizes:
        grps.append((_k0, _k0 + _sz))
        _k0 += _sz
    assert _k0 == KT
    maxg = max(k1 - k0 for k0, k1 in grps)

    nc = bacc.Bacc(
        "TRN2",
        target_bir_lowering=False,
        debug=False,
        enable_asserts=False,
        num_devices=1 if nocc else NCORES,
    )

    a = nc.dram_tensor("a", [KT, 128, SP], u8, kind="ExternalInput").ap()
    x0 = nc.dram_tensor("x0", [128, NPAD], f16, kind="ExternalInput").ap()
    # per-(bank chunk, partition) dequant scales; the host sorts dst
    # columns by quant range so each (chunk, partition) slot's 4 columns
    # share one scale -> the tanh eviction is 3 bank-wide activations
    csc = nc.dram_tensor("csc", [128, 3], f32, kind="ExternalInput").ap()
    # broadcast W1 row tiled x4 (only read when not w1_ones)
    w1b = nc.dram_tensor("w1b", [128, 512], f16, kind="ExternalInput").ap()
    out = nc.dram_tensor("out", [128, SP], f32, kind="ExternalOutput").ap()

    with tile.TileContext(nc) as tc:
        with (
            tc.tile_pool(name="xp", bufs=1) as xp,
            tc.tile_pool(name="a8p", bufs=1) as a8p,
            tc.tile_pool(name="fc", bufs=ncast) as fcp,
            tc.tile_pool(name="ps", bufs=1, space="PSUM") as ps,
            tc.tile_pool(name="dr", bufs=1, space="DRAM") as dr,
        ):
            # x0 is dead once layer 1 finishes; share one slot for both
            x0s = xp.tile([128, NPAD], f16, tag="xs")
            x1s = xp.tile([128, NPAD], f16, tag="xs")
            cscs = xp.tile([128, 3], f32, tag="cscs")
            w1s = xp.tile([128, 512], f16, tag="w1s")
            zl = xp.tile([128, 512], f16, tag="zl")
            warm = xp.tile([128, 1], f32, tag="warm")
            nc.scalar.dma_start(cscs[:], csc)
            if not w1_ones:
                nc.scalar.dma_start(w1s[:], w1b)
            nc.vector.memset(zl[:], 0.0)
            # pre-load the ACT tanh table so the layer-1 eviction doesn't
            # pay the table load on the critical path
            nc.scalar.activation(
                warm[:], zl[:, 0:1], mybir.ActivationFunctionType.Tanh
            )

            agin = dr.tile([128, SP], f16)
            agout = dr.tile([NCORES * 128, SP], f16, addr_space="Shared")

            a8_tiles = {}

            def cast_group(gi, engines="vap", deps=None):
                """u8 -> f16 of resident group gi, split over the engines in
                `engines` (v=DVE, a=ACT in ~1us slices, p=POOL), shares
                proportional to their elementwise rates.  `deps` maps an
                engine letter to instruction names the slice must follow —
                used at the layer boundary so the scheduler cannot hoist
                casts ahead of the tanh -> AllGather chain."""
                k0, k1 = grps[gi]
                w = (k1 - k0) * SP
                a8 = a8_tiles[gi]
                fb = fcp.tile([128, maxg * SP], f16, tag="fc")
                rates = {"v": 4, "a": 5, "p": 3}
                tot = sum(rates[e] for e in engines)
                halves = deps.pop("halves", 1) if deps else 1
                bounds = [(w * h) // halves for h in range(halves + 1)]
                for h0, h1 in zip(bounds, bounds[1:]):
                  wh = h1 - h0
                  c0 = h0
                  for e in engines:
                    c1 = h1 if e == engines[-1] else c0 + (wh * rates[e]) // tot
                    insts = []
                    if e == "v":
                        insts.append(
                            nc.vector.tensor_copy(fb[:, c0:c1], a8[:, c0:c1])
                        )
                    elif e == "a":
                        # <=1.3k-elem slices so ACT never blocks the layer-1
                        # eviction chain behind a long copy
                        s0 = c0
                        while s0 < c1:
                            s1 = min(s0 + 1280, c1)
                            insts.append(
                                nc.scalar.copy(fb[:, s0:s1], a8[:, s0:s1])
                            )
                            s0 = s1
                    else:
                        insts.append(
                            nc.gpsimd.tensor_copy(fb[:, c0:c1], a8[:, c0:c1])
                        )
                    if deps and e in deps:
                        for inst in insts:
                            inst.ins.add_sync_dependencies_from(_NameSet(deps[e]))
                    c0 = c1
                return fb

            # ---- layer 1 (A-stationary; psum is [dst slot, feat]) ----
            # one psum tile per 2KiB bank: psum reads are dependency-tracked
            # whole-tile, so per-bank tiles let each bank's eviction start at
            # its own stop instead of after the layer's last matmul
            psum1 = []
            for ci, (c0, cn) in enumerate(CHUNKS):
                p1t = ps.tile([128, cn], f32, tag=f"acc1_{ci}", name=f"p1_{ci}")
                psum1.append(p1t)
            for ci, (c0, cn) in enumerate(CHUNKS):
                nc.tensor.matmul(
                    psum1[ci][:, 0:cn], zl[:, 0:128], zl[:, 0:cn],
                    start=True, stop=False,
                )
            # x0 for the first four groups rides ahead of their a8 loads so
            # the DMA queue can stay a couple of groups in front of the PE
            xlead = grps[3][1] * 128
            for gi, (k0, k1) in enumerate(grps):
                a8 = a8p.tile([128, (k1 - k0) * SP], u8, tag=f"a8_{gi}")
                a8_tiles[gi] = a8
                kb = {k0, k1}
                if 3 <= gi <= 6 and (k1 - k0) % 2 == 0:
                    kb.add((k0 + k1) // 2)
                for k in range(k0, k1 + 1):
                    if k % 10 in (8, 9) and k0 < k < k1 and _rows(k - 1) != \
                            NPADROW:
                        kb.add(k)
                    if k % 10 == 0 and k0 < k < k1 and _rows(k - 1) == \
                            NPADROW:
                        kb.add(k)
                kb = sorted(kb)
                for b0, b1 in zip(kb, kb[1:]):
                    nr = NPADROW if _rows(b0) == NPADROW else 128
                    nc.sync.dma_start(
                        a8[0:nr, (b0 - k0) * SP:(b1 - k0) * SP].rearrange(
                            "p (k j) -> p k j", k=b1 - b0
                        ),
                        a[b0:b1, 0:nr].rearrange("k p j -> p k j"),
                    )
                if gi == 0:
                    nc.sync.dma_start(x0s[:, 0:xlead], x0[:, 0:xlead])
                if gi >= 4:
                    nc.sync.dma_start(
                        x0s[:, k0 * 128:k1 * 128], x0[:, k0 * 128:k1 * 128]
                    )
            ng = len(grps)
            for oi, gi in enumerate(range(ng)):
                k0, k1 = grps[gi]
                fb = cast_group(gi, deps={"halves": 2}
                                if 3 <= gi <= 6 else None)
                if oi < ng - 1:
                    for k in range(k0, k1):
                        kk = k - k0
                        nr = _rows(k)
                        rhs = x0s[0:nr, k * 128:(k + 1) * 128]
                        for t in range(10):
                            ci, tt = (t // 4, t % 4)
                            nc.tensor.matmul(
                                psum1[ci][:, tt * 128:(tt + 1) * 128],
                                fb[0:nr, kk * SP + t * 128:
                                   kk * SP + (t + 1) * 128],
                                rhs,
                                start=False, stop=False,
                            )
                else:
                    # final group t-outer: each dst range finishes early so
                    # the tanh eviction overlaps the remaining matmuls
                    for t in range(10):
                        ci, tt = (t // 4, t % 4)
                        for k in range(k0, k1):
                            kk = k - k0
                            nr = _rows(k)
                            last_mm = nc.tensor.matmul(
                                psum1[ci][:, tt * 128:(tt + 1) * 128],
                                fb[0:nr, kk * SP + t * 128:
                                   kk * SP + (t + 1) * 128],
                                x0s[0:nr, k * 128:(k + 1) * 128],
                                start=False,
                                stop=(k == k1 - 1 and t in (3, 7, 9)),
                            )

            # evict layer 1: x1 = tanh(cs_dst * psum1) [* W1] on ACT; DMA to
            # the AllGather bounce per psum bank so agin lands early.  The
            # whole tanh -> agin -> AllGather -> x1s chain is the only work
            # between the two PE-bound layers, so it runs at high priority
            # and its DMAs ride the otherwise-idle SP queue.
            agin_sb = xp.tile([128, SP], f16, tag="agin")
            # keep-warm matmuls: PE would otherwise idle across the AllGather
            # valley and restart cold (2.4x slower for the first 3us)
            psumd = ps.tile([128, 512], f32, tag="warmups")
            for _ in range(ndummy):
                dmm = nc.tensor.matmul(
                    psumd[:], zl[:, 0:128], zl[:, 0:512],
                    start=True, stop=True, skip_group_check=True,
                )
                # pin behind layer 1 so the scheduler cannot hoist the
                # warm-up matmuls to the (DMA-bound) start of the program
                dmm.ins.add_sync_dependencies_from(_NameSet([last_mm.ins.name]))
            tanh_last = None
            with tc.high_priority():
                for ci, (c0, cn) in enumerate(CHUNKS):
                    tanh_last = nc.scalar.activation(
                        agin_sb[:, c0:c0 + cn], psum1[ci][:, 0:cn],
                        mybir.ActivationFunctionType.Tanh,
                        scale=cscs[:, ci:ci + 1],
                    )
                    if not w1_ones:
                        nc.vector.tensor_mul(
                            agin_sb[:, c0:c0 + cn], agin_sb[:, c0:c0 + cn],
                            w1s[:, 0:cn]
                        )
                    nc.sync.dma_start(
                        agin[:, c0:c0 + cn], agin_sb[:, c0:c0 + cn]
                    )

                if nocc:
                    ag_inst = nc.sync.dma_start(agout[0:128, :], agin[:])
                else:
                    ag_inst = nc.gpsimd.collective_compute(
                        "AllGather",
                        mybir.AluOpType.bypass,
                        replica_groups=[list(range(NCORES))],
                        ins=[agin.opt()],
                        outs=[agout.opt()],
                    )
                # A's src row-blocks are rotated per core so block 0 is the
                # core's OWN rank: layer 2's first 10 k-tiles read agin_sb
                # directly (no AllGather round-trip), and block i (i>=1) is
                # rank (pid+i)%8, fetched from agout at a register-computed
                # offset.
                agdep = _NameSet([ag_inst.ins.name])
                pid = nc.sync.partition_id()
                x1dmas = []
                for i in range(1, NCORES):
                    boff = ((pid + i) % NCORES) * (128 * SP)
                    if i == 1:
                        # lead slice so k-tile 10 can start while the rest
                        # of the block is in flight
                        dsrc = agout[0:128, 0:384].copy()
                        dsrc.offset = boff
                        x1dmas.append(
                            nc.sync.dma_start(x1s[:, SP:SP + 384], dsrc)
                        )
                        dsrc = agout[0:128, 384:SP].copy()
                        dsrc.offset = boff + 384
                        x1dmas.append(
                            nc.sync.dma_start(
                                x1s[:, SP + 384:2 * SP], dsrc
                            )
                        )
                    else:
                        dsrc = agout[0:128, :].copy()
                        dsrc.offset = boff
                        x1dmas.append(
                            nc.sync.dma_start(
                                x1s[:, i * SP:(i + 1) * SP], dsrc
                            )
                        )
                # DRAM->SBUF reads of the collective output are not tracked
                # as data deps in the single-core twin; pin them so the
                # scheduler cannot float them ahead of the agin writes
                for d in x1dmas:
                    d.ins.add_sync_dependencies_from(agdep)

            # ---- layer 2 (X-stationary; psum is [feat, dst]) ----
            # All of A is already resident as u8; only the casts re-run.
            # The first ring of casts has no x1 dependency, so it completes
            # during the AllGather and PE starts as soon as rank 0 lands.
            psum2 = []
            for ci, (c0, cn) in enumerate(CHUNKS):
                p2t = ps.tile([128, cn], f32, tag=f"acc2_{ci}", name=f"p2_{ci}")
                psum2.append(p2t)
            ob = xp.tile([128, SP], f32, tag="ob")

            def lhsT_of(k):
                # row-block 0 is the core's own rank: its activations are
                # already on-chip in agin_sb (same [dst slot, feat] layout)
                nr = _rows(k)
                if k < 10:
                    return agin_sb[0:nr, k * 128:(k + 1) * 128]
                return x1s[0:nr, k * 128:(k + 1) * 128]

            first = True
            tdep = [tanh_last.ins.name]
            adep = [ag_inst.ins.name]
            for gi, (k0, k1) in enumerate(grps):
                # keep ACT free for the tanh chain and POOL free for the
                # AllGather issue while the boundary groups pre-cast on DVE;
                # ACT/POOL rejoin once their part of the chain retires
                if gi < 3:
                    fb = cast_group(gi, "v")
                elif gi < 5:
                    fb = cast_group(gi, "va", deps={"a": tdep})
                elif gi < 9:
                    fb = cast_group(gi, "vap", deps={"a": tdep, "p": adep})
                else:
                    fb = cast_group(gi)
                last_grp = gi == len(grps) - 1
                if not last_grp:
                    for k in range(k0, k1):
                        kk = k - k0
                        lhsT = lhsT_of(k)
                        nr = _rows(k)
                        for ci, (c0, cn) in enumerate(CHUNKS):
                            nc.tensor.matmul(
                                psum2[ci][:, 0:cn],
                                lhsT,
                                fb[0:nr, kk * SP + c0: kk * SP + c0 + cn],
                                start=first, stop=False,
                            )
                        first = False
                else:
                    # final group: bank-outer with per-bank stops; ALL
                    # evictions are created after the matmuls (psum reads
                    # are tracked whole-tile, so an earlier-created read
                    # would falsely serialize the later banks' matmuls)
                    for ci, (c0, cn) in enumerate(CHUNKS):
                        for k in range(k0, k1):
                            kk = k - k0
                            nc.tensor.matmul(
                                psum2[ci][:, 0:cn],
                                lhsT_of(k),
                                fb[0:_rows(k), kk * SP + c0:
                                   kk * SP + c0 + cn],
                                start=False, stop=(k == k1 - 1),
                            )
                    # GPSIMD cannot read PSUM on HW: evict banks on DVE/ACT
                    dq_eng = (nc.sync, nc.scalar, nc.sync)
                    with tc.high_priority():
                        for ci, (c0, cn) in enumerate(CHUNKS):
                            if ci == 1:
                                nc.scalar.copy(
                                    ob[:, c0:c0 + cn], psum2[ci][:, 0:cn]
                                )
                            else:
                                nc.vector.tensor_copy(
                                    ob[:, c0:c0 + cn], psum2[ci][:, 0:cn]
                                )
                            dq_eng[ci].dma_start(
                                out[:, c0:c0 + cn], ob[:, c0:c0 + cn]
                            )

    nc.compile()
    return nc


def get_program(nocc=False, gsizes=GSIZES, ncast=NCAST, w1_ones=True,
                ndummy=NDUMMY):
    key = ("nc", nocc, tuple(gsizes), ncast, w1_ones, ndummy)
    if key not in _PROG_CACHE:
        _PROG_CACHE[key] = _build_program(nocc, gsizes, ncast, w1_ones,
                                          ndummy)
    return _PROG_CACHE[key]


def _slot_order():
    """Slot s = t*128 + p (tile t in 0..9, partition p) listed in quant-sort
    order: chunks of 4 (banks 0/1) or 2 (bank 2) consecutive sorted columns
    share one (chunk, partition) slot group, hence one dequant scale."""
    slots = np.empty(SP, np.int64)
    i = 0
    for ci, (tile0, ntile) in enumerate(((0, 4), (4, 4), (8, 2))):
        for p in range(128):
            for ti in range(ntile):
                slots[i] = (tile0 + ti) * 128 + p
                i += 1
    assert i == SP
    return slots


_SLOTS = _slot_order()


def _core_perm(colmax_ext):
    """perm[s] = original local dst column (or >=S for pad) in slot s, with
    columns sorted by quant range so slot groups share a scale."""
    order = np.argsort(-colmax_ext, kind="stable")  # [SP] sorted col ids
    perm = np.empty(SP, np.int64)
    perm[_SLOTS] = order
    return perm


def build_in_maps(x, src, dst, vals, W):
    """Host-side prep: dense A^T shard (u8 quantized, 4 sorted columns per
    scale group) + x0, both in the per-core permuted slot order."""
    import scipy.sparse as sp

    x = np.asarray(x, np.float32)
    src = np.asarray(src, np.int64)
    dst = np.asarray(dst, np.int64)
    vals = np.asarray(vals, np.float32)
    W = np.asarray(W, np.float32)

    # A[dst, src] = sum of vals  ->  we build AT[src, dst]
    AT = sp.coo_matrix((vals, (src, dst)), shape=(N, N)).toarray()

    # per-core column permutations (dst side of A, src rows of A, x rows)
    perms = []
    steps = []
    cscs = []
    for c in range(NCORES):
        ATc = AT[:, c * S:(c + 1) * S]  # [N, S] float32
        colmax_ext = np.full(SP, -1.0, np.float32)
        colmax_ext[:S] = ATc.max(axis=0)
        perm = _core_perm(colmax_ext)
        # group scale = max colmax over each slot group (same (chunk, p))
        cm_slot = np.maximum(colmax_ext[perm], 1e-9)  # [SP] by slot
        step_slot = np.empty(SP, np.float32)
        csc = np.empty((128, 3), np.float32)
        for ci, (tile0, ntile) in enumerate(((0, 4), (4, 4), (8, 2))):
            t_sl = slice(tile0 * 128, (tile0 + ntile) * 128)
            cm = cm_slot[t_sl].reshape(ntile, 128)    # [ntile, p]
            gmax = cm.max(axis=0) / 255.0             # [p]
            csc[:, ci] = gmax
            step_slot[t_sl] = np.tile(gmax[None, :], (ntile, 1)).reshape(-1)
        perms.append(perm)
        steps.append(step_slot)
        cscs.append(np.ascontiguousarray(csc))

    # per-core src slot -> node mapping: row-block i of core c is rank
    # (c+i)%8 (own rank first, so layer 2 starts from on-chip activations),
    # permuted within the block by that rank's own column permutation
    node2s, valid2s = [], []
    for c in range(NCORES):
        node2 = np.empty(NPAD, np.int64)
        valid2 = np.empty(NPAD, bool)
        for i in range(NCORES):
            r = (c + i) % NCORES
            pr = perms[r]
            valid = pr < S
            node2[i * SP:(i + 1) * SP] = np.where(valid, r * S + pr, 0)
            valid2[i * SP:(i + 1) * SP] = valid
        node2s.append(node2)
        valid2s.append(valid2)

    xw = x * W[0][None, :]

    w1brow = np.ascontiguousarray(
        np.tile(W[1][None, :], (128, 4))
    ).astype(np.float16)

    in_maps = []
    for c in range(NCORES):
        node2, valid2 = node2s[c], valid2s[c]
        x0p = np.zeros((NPAD, D), np.float32)
        x0p[valid2] = xw[node2[valid2]]
        x0h = np.ascontiguousarray(
            x0p.reshape(KT, 128, D).transpose(1, 0, 2).reshape(128, KT * D)
        ).astype(np.float16)
        ATc = AT[:, c * S:(c + 1) * S]  # [N, S] float32
        perm = perms[c]
        valid = perm < S
        ATs = np.zeros((N, SP), np.float32)
        ATs[:, valid] = ATc[:, perm[valid]]           # columns in slot order
        Aq = np.clip(np.rint(ATs / steps[c][None, :]), 0, 255).astype(
            np.uint8
        )
        Ap = np.zeros((NPAD, SP), Aq.dtype)
        Ap[valid2] = Aq[node2[valid2]]                # rows in slot order
        a3 = np.ascontiguousarray(Ap.reshape(KT, 128, SP))
        in_maps.append(
            {
                "a": a3,
                "x0": x0h,
                "csc": cscs[c],
                "w1b": w1brow,
            }
        )
    return in_maps, (steps, perms)


def assemble_output(results, aux):
    steps, perms = aux
    outs = []
    for c in range(NCORES):
        ot = np.asarray(results[c]["out"], np.float32)  # [128, SP] feat-major
        ot = ot * steps[c][None, :]  # per-dst dequant (layer-2)
        perm = perms[c]
        valid = perm < S
        o = np.zeros((S, 128), np.float32)
        o[perm[valid]] = ot[:, valid].T             # un-permute dst slots
        outs.append(o)
    return np.ascontiguousarray(np.concatenate(outs, axis=0))


def kernel(x, src, dst, vals, W):
    from concourse import bass_utils

    w1_ones = bool(np.all(np.asarray(W)[1] == 1.0))
    nc = get_program(w1_ones=w1_ones)
    in_maps, steps = build_in_maps(x, src, dst, vals, W)
    # The axon terminal can wedge when a different program was loaded
    # earlier in its lifetime; after the crash the terminal restarts and a
    # retry succeeds.  Back off progressively to ride out the restart.
    import time as _time

    last_err = None
    for sleep_s in (10.0, 30.0, 60.0, 0.0):
        try:
            res = bass_utils.run_bass_kernel_spmd(
                nc, in_maps, core_ids=list(range(NCORES))
            )
            return assemble_output(res.results, steps)
        except Exception as e:  # noqa: BLE001
            last_err = e
            _time.sleep(sleep_s)
    raise last_err
